# revision 26
# baseline (speedup 1.0000x reference)
"""Trainium2 Bass kernel for the PCNN (piecewise-CNN) bag-classification model.

v2b pipeline (per NeuronCore, data-parallel over sentences, 256/core):
  1. ONE batched indirect-DMA gather per 32-sentence block (4096 fp8 embedding
     rows; 8 gpsimd instructions/core vs 256 -> SWDGE fixed cost ~19us not 280)
  2. PE transposes (fp8) -> channel-major xcq [128, 4, 4112]; chunk 3 is a
     +15-column-shifted replica of chunk 2 (SBUF->SBUF DMA) so its tap views
     are 16-byte aligned for DoubleRow pairing
  3. conv1d(k=3, edge-pad) as 4 fp8 DoubleRow pair-matmuls (256-deep
     contraction, 0.5 cyc/row) + 1 plain fp8 matmul per (subgroup, filter
     chunk); +128*mask piece-0 bias rides the center-tap view's row 54
  4. PCNN piecewise max-pool: two fp8 rank-1 increment matmuls (pieces 1,2);
     DVE reduce_max reads both filter-chunk PSUM banks in one op
  5. ReLU(max-128+conv_b), dense to 53 logits, bag segment-mean matmul
  6. ReduceScatter(add) over 8 cores -> each core softmaxes its 32 bags,
     host concatenates the 8 x [32,53] slices
"""

import os
import sys

for _p in ("/opt/trn_rl_repo",):
    if _p not in sys.path:
        sys.path.insert(0, _p)

import numpy as np
import ml_dtypes

# ---------------- problem constants (hardcoded per spec) ----------------
N = 2048          # total sentences
L = 120           # max sentence length
LP = 122          # edge-padded length
NCORES = 8
NS = N // NCORES  # 256 sentences per core
BLK = 32          # sentences per block
NBLK = NS // BLK  # 8 blocks
SGS = 4           # sentences per matmul subgroup
SG_PER_BLK = BLK // SGS          # 8
SG_COLS = 512                    # padded columns per subgroup (4*122=488 real)
BLK_COLS = SG_PER_BLK * SG_COLS  # 4096
TILES_PER_BLK = BLK_COLS // 128  # 32
NF = 230
NREL = 53
NBAGS = 256
BAGS_PER_CORE = NBAGS // NCORES  # 32
VOCAB = 100000
WD = 300
PD = 5
ELEM = 300        # gathered bf16 row length == table row stride (indirect
                  # gather's index coefficient comes from the src shape)
BMASK = 128.0     # piecewise-pool mask bias (exact in fp8/bf16)
FCH = [(0, 128), (128, 102)]     # filter chunks
CCH = [(0, 128), (128, 128), (256, 44)]  # word-channel transpose chunks
CSTR = 4112       # xcq chunk stride (4096 cols + room for the +15 replica)
RSHIFT = 15
# DoubleRow k-tile pair views (chunk, tap); pair-dim step must be %16 fp8
# bytes, so taps pair across chunks (step 4112) and chunk2 tap1 reads the
# +15-shifted replica in chunk slot 3 (step 4128). (2,2) runs as a plain
# fp8 single. The PCNN mask row (channel 310 = chunk2 partition 54, stored
# center-tap aligned) gets weight +BMASK only in the tap-1 view.
PAIRS = [((0, 0), (1, 0)), ((0, 1), (1, 1)), ((0, 2), (1, 2)),
         ((2, 0), (3, 1))]
SINGLE = (2, 2)

BF16 = ml_dtypes.bfloat16
FP8 = ml_dtypes.float8_e4m3

_PROGRAM = None
LAST_RESULT = None


def _view_off(v):
    """fp8 free-dim offset of a k-tile view within xcq [128, 4, CSTR]."""
    c, k = v
    if c == 3:  # chunk2 replica, shifted by RSHIFT
        return 3 * CSTR + RSHIFT + k
    return c * CSTR + k


def _build_program():
    import concourse.bass as bass
    import concourse.mybir as mybir
    import concourse.tile as tile
    from concourse import bacc

    f32 = mybir.dt.float32
    bf16 = mybir.dt.bfloat16
    fp8 = mybir.dt.float8e4
    i32 = mybir.dt.int32
    AF = mybir.ActivationFunctionType
    AX = mybir.AxisListType
    PM = mybir.MatmulPerfMode

    nc = bacc.Bacc(
        "TRN2",
        target_bir_lowering=False,
        debug=False,
        num_devices=NCORES,
    )

    # ------------- external I/O -------------
    wemb = nc.dram_tensor("wemb", [VOCAB, ELEM], bf16, kind="ExternalInput").ap()
    idxw_d = nc.dram_tensor("idxw", [128, NBLK * TILES_PER_BLK], i32,
                            kind="ExternalInput").ap()
    xpf_d = nc.dram_tensor("xpf", [NBLK, 11, BLK_COLS], fp8, kind="ExternalInput").ap()
    masks_d = nc.dram_tensor("masksd", [NBLK, 4, BLK_COLS], fp8,
                             kind="ExternalInput").ap()
    snorm_d = nc.dram_tensor("snorm", [NS, NBAGS], bf16, kind="ExternalInput").ap()
    wtp_d = nc.dram_tensor("wtp", [128, 2560], fp8, kind="ExternalInput").ap()
    selb_d = nc.dram_tensor("selb", [4, 2 * 128], fp8, kind="ExternalInput").ap()
    dwt_d = nc.dram_tensor("dwt", [128, 6 * NREL], bf16, kind="ExternalInput").ap()
    actb_d = nc.dram_tensor("actb", [128, 2], f32, kind="ExternalInput").ap()
    dbias_d = nc.dram_tensor("dbias", [1, NREL], bf16, kind="ExternalInput").ap()
    id16_d = nc.dram_tensor("id16d", [128, 128], bf16, kind="ExternalInput").ap()
    out_d = nc.dram_tensor("out", [BAGS_PER_CORE, NREL], f32, kind="ExternalOutput").ap()
    debug = bool(int(os.environ.get("KERNEL_DEBUG", "0")))
    if debug:
        dbg_xcq = nc.dram_tensor("dbg_xcq", [128, 4, CSTR], fp8,
                                 kind="ExternalOutput").ap()
        dbg_pooled = nc.dram_tensor("dbg_pooled", [128, 2, 3, NS], f32,
                                    kind="ExternalOutput").ap()
        dbg_bag = nc.dram_tensor("dbg_bag", [NBAGS, NREL], f32,
                                 kind="ExternalOutput").ap()

    with tile.TileContext(nc) as tc:
        import contextlib

        ctx = contextlib.ExitStack()
        with ctx:
            singles = ctx.enter_context(tc.tile_pool(name="singles", bufs=1))

            # persistent tiles
            wtp_sb = singles.tile([128, 2, 4, 2, 128], fp8, name="wtp")
            wts_sb = singles.tile([128, 2, 128], fp8, name="wts")
            selb_sb = singles.tile([4, 2, 128], fp8, name="selb")
            snorm_sb = [singles.tile([128, NBAGS], bf16, name=f"sn{c}") for c in range(2)]
            idxw_sb = singles.tile([128, NBLK * TILES_PER_BLK], i32, name="idx")
            dwt_sb = singles.tile([128, 6 * NREL], bf16)
            actb_sb = singles.tile([128, 2], f32)
            dbias_sb = singles.tile([1, NREL], bf16)
            id16 = singles.tile([128, 128], bf16, name="id16")
            ones_sb = singles.tile([1, 128], bf16)
            pooled = singles.tile([128, 2, 3, NS], f32, name="pool")

            nc.sync.dma_start(out=wtp_sb[:, :, :, :, :], in_=wtp_d[:, 0:2048])
            nc.sync.dma_start(out=wts_sb[:, :, :], in_=wtp_d[:, 2048:2304])
            nc.sync.dma_start(out=selb_sb[:, :, :], in_=selb_d[:, :])
            for c in range(2):
                nc.sync.dma_start(out=snorm_sb[c][:, :], in_=snorm_d[c * 128:(c + 1) * 128, :])
            nc.sync.dma_start(out=idxw_sb[:, :], in_=idxw_d[:, :])
            nc.sync.dma_start(out=dwt_sb[:, :], in_=dwt_d[:, :])
            nc.sync.dma_start(out=actb_sb[:, :], in_=actb_d[:, :])
            nc.sync.dma_start(out=dbias_sb[:, :], in_=dbias_d[:, :])
            nc.sync.dma_start(out=id16[:, :], in_=id16_d[:, :])
            nc.vector.memset(ones_sb[:, :], 1.0)

            xg_pool = ctx.enter_context(tc.tile_pool(name="xg", bufs=2))
            xcq_pool = ctx.enter_context(tc.tile_pool(name="xcq", bufs=2))
            mask_pool = ctx.enter_context(tc.tile_pool(name="mask", bufs=2))
            tp_psum = ctx.enter_context(tc.tile_pool(name="tp", bufs=2, space="PSUM"))
            cv_psum = ctx.enter_context(tc.tile_pool(name="cv", bufs=3, space="PSUM"))

            NCV = SGS * LP  # 488 contiguous conv output columns per subgroup

            def emit_conv(blk, sg, xcq, ps):
                # fw padded to 128 (zero weight cols) so every PSUM row in both
                # banks is written; out-partition count doesn't affect PE time.
                # chunk2/3 rows 55:128 are never written, so those views
                # contract only K=55 partitions (same math: weights were 0).
                for fc in range(2):
                    out_ap = ps[0:128, fc, 0:NCV]
                    for p, (vA, vB) in enumerate(PAIRS):
                        kp = 55 if vA[0] >= 2 else 128
                        rb = xcq[0:kp, 0, 0:1]
                        offA = _view_off(vA) + sg * SG_COLS
                        delta = _view_off(vB) - _view_off(vA)
                        rhs = bass.AP(
                            tensor=rb.tensor, offset=rb.offset + offA,
                            ap=[rb.ap[0], [delta, 2], [1, NCV]],
                        )
                        nc.tensor.matmul(
                            out=out_ap,
                            lhsT=wtp_sb[0:kp, fc, p, :, 0:128],
                            rhs=rhs,
                            start=(p == 0),
                            stop=False,
                            perf_mode=PM.DoubleRow,
                            skip_group_check=True,
                        )
                    rb = xcq[0:55, 0, 0:1]
                    offS = _view_off(SINGLE) + sg * SG_COLS
                    rhs = bass.AP(tensor=rb.tensor, offset=rb.offset + offS,
                                  ap=[rb.ap[0], [1, NCV]])
                    nc.tensor.matmul(
                        out=out_ap,
                        lhsT=wts_sb[0:55, fc, 0:128],
                        rhs=rhs,
                        start=False,
                        stop=False,
                        skip_group_check=True,
                    )

            def emit_jphases(blk, sg, mask_sb, ps):
                s0 = blk * BLK + sg * SGS
                for j in range(3):
                    pb0 = ps[0:128, 0, 0:1]
                    rin = bass.AP(
                        tensor=pb0.tensor, offset=pb0.offset,
                        ap=[pb0.ap[0], [SG_COLS, 2], [LP, SGS], [1, L]],
                    )
                    pb = pooled[0:128, 0, j, s0:s0 + SGS]
                    rout = bass.AP(tensor=pb.tensor, offset=pb.offset,
                                   ap=[pb.ap[0], [3 * NS, 2], [1, SGS]])
                    nc.vector.reduce_max(out=rout, in_=rin, axis=AX.X)
                    if j < 2:
                        for fc in range(2):
                            nc.tensor.matmul(
                                out=ps[0:128, fc, 0:NCV],
                                lhsT=selb_sb[0:4, j, 0:128],
                                rhs=mask_sb[0:4, sg * SG_COLS:sg * SG_COLS + NCV],
                                start=False,
                                stop=(j == 1),
                                skip_group_check=True,
                            )

            pending = []
            for blk in range(NBLK):
                # ---- batched gather (token-major, fp8) ----
                xg = xg_pool.tile([128, TILES_PER_BLK, ELEM], bf16, tag="xg")
                for t in range(TILES_PER_BLK):
                    col = blk * TILES_PER_BLK + t
                    nc.gpsimd.indirect_dma_start(
                        out=xg[:, t, 0:WD],
                        out_offset=None,
                        in_=wemb[:, 0:WD],
                        in_offset=bass.IndirectOffsetOnAxis(
                            ap=idxw_sb[:, col:col + 1], axis=0),
                    )
                mask_sb = mask_pool.tile([4, BLK_COLS], fp8, tag="mask")
                nc.sync.dma_start(out=mask_sb[:, :], in_=masks_d[blk, :, :])

                # ---- transpose to channel-major (fp8) ----
                xcq = xcq_pool.tile([128, 4, CSTR], fp8, tag="xcq")
                nc.sync.dma_start(out=xcq[44:55, 2, 0:BLK_COLS], in_=xpf_d[blk, :, :])
                for grp in range(4):  # 8 token-tiles per group
                    for cc, (c0, pw) in enumerate(CCH):
                        tpA = tp_psum.tile([128, 4, 128], bf16, tag="tp", name=f"tpA{cc}")
                        tpB = tp_psum.tile([128, 4, 128], bf16, tag="tp", name=f"tpB{cc}")
                        for t in range(8):
                            ti = grp * 8 + t
                            tgt = tpA if t % 2 == 0 else tpB
                            nc.tensor.transpose(
                                out=tgt[0:pw, t // 2, :],
                                in_=xg[:, ti, c0:c0 + pw],
                                identity=id16[:, :],
                            )
                        for half, tp in ((0, tpA), (1, tpB)):
                            cb = xcq[0:pw, cc, grp * 1024 + half * 128:
                                     grp * 1024 + half * 128 + 1]
                            dst = bass.AP(
                                tensor=cb.tensor, offset=cb.offset,
                                ap=[cb.ap[0], [256, 4], [1, 128]],
                            )
                            nc.scalar.copy(out=dst, in_=tp[0:pw, :, :])

                # chunk2 -> +RSHIFT replica in chunk slot 3 (covers word rows
                # 0:44, pf rows 44:54 and the mask row 54)
                nc.sync.dma_start(out=xcq[0:55, 3, RSHIFT:RSHIFT + BLK_COLS],
                                  in_=xcq[0:55, 2, 0:BLK_COLS])
                if debug and blk == 0:
                    nc.sync.dma_start(out=dbg_xcq[:, :, :], in_=xcq[:, :, :])

                for sg in range(SG_PER_BLK):
                    ps = cv_psum.tile([128, 2, SG_COLS], f32, tag="cv",
                                      name=f"cv{blk}_{sg}")
                    emit_conv(blk, sg, xcq, ps)
                    if pending:
                        emit_jphases(*pending.pop(0))
                    pending.append((blk, sg, mask_sb, ps))

            while pending:
                emit_jphases(*pending.pop(0))

            # ---------------- tail ----------------
            if debug:
                nc.sync.dma_start(out=dbg_pooled[:, :, :, :], in_=pooled[:, :, :, :])
            pr = [singles.tile([128, 3, NS], bf16, name=f"pr{c}") for c in range(2)]
            for fc in range(2):
                nc.scalar.activation(
                    out=pr[fc][:, :, :],
                    in_=pooled[:, fc, :, :],
                    func=AF.Relu,
                    bias=actb_sb[:, fc:fc + 1],
                    scale=1.0,
                )

            # dense: logitsT [53, 256] = sum_{j,fc} dwt[(j,fc)].T @ pr
            lg_ps = cv_psum.tile([NREL, NS], f32, tag="cv", name="lgps")
            nmm = 0
            for j in range(3):
                for fc, (f0, fw) in enumerate(FCH):
                    nc.tensor.matmul(
                        out=lg_ps[:, :],
                        lhsT=dwt_sb[0:fw, (j * 2 + fc) * NREL:(j * 2 + fc + 1) * NREL],
                        rhs=pr[fc][0:fw, j, :],
                        start=(nmm == 0),
                        stop=(nmm == 5),
                    )
                    nmm += 1
            lg_sb = singles.tile([NREL, NS], bf16)
            nc.vector.tensor_copy(out=lg_sb[:, :], in_=lg_ps[:, :])

            # transpose logits -> per-sentence rows [256, 53]
            ls = [singles.tile([128, NREL], bf16, name=f"ls{c}") for c in range(2)]
            for sc in range(2):
                ltp = cv_psum.tile([128, NREL], bf16, tag="cv", name="ltp")
                nc.tensor.transpose(
                    out=ltp[0:128, 0:NREL],
                    in_=lg_sb[:, sc * 128:(sc + 1) * 128],
                    identity=id16[0:NREL, 0:NREL],
                )
                nc.vector.tensor_copy(out=ls[sc][:, :], in_=ltp[0:128, 0:NREL])

            # bag aggregation (+ dense bias/8), full 256 bags of partials
            cc_dram = ctx.enter_context(tc.tile_pool(name="ccd", bufs=1, space="DRAM"))
            cc_in = cc_dram.tile([NBAGS, NREL], f32)
            cc_out = cc_dram.tile([BAGS_PER_CORE, NREL], f32)
            for bc in range(2):
                bg = cv_psum.tile([128, NREL], f32, tag="cv", name="bg")
                for sc in range(2):
                    nc.tensor.matmul(
                        out=bg[:, :],
                        lhsT=snorm_sb[sc][:, bc * 128:(bc + 1) * 128],
                        rhs=ls[sc][:, :],
                        start=(sc == 0),
                        stop=False,
                    )
                nc.tensor.matmul(
                    out=bg[:, :],
                    lhsT=ones_sb[0:1, 0:128],
                    rhs=dbias_sb[0:1, :],
                    start=False,
                    stop=True,
                )
                bg_sb = singles.tile([128, NREL], f32, name=f"bgs{bc}")
                nc.vector.tensor_copy(out=bg_sb[:, :], in_=bg[:, :])
                nc.sync.dma_start(out=cc_in[bc * 128:(bc + 1) * 128, :], in_=bg_sb[:, :])

            if debug:
                nc.sync.dma_start(out=dbg_bag[:, :], in_=cc_in[:, :])
            nc.gpsimd.collective_compute(
                "ReduceScatter",
                mybir.AluOpType.add,
                replica_groups=[list(range(NCORES))],
                ins=[cc_in.opt()],
                outs=[cc_out.opt()],
            )

            # softmax over the 53 relations for this core's 32 bags
            t = singles.tile([BAGS_PER_CORE, NREL], f32, name="sm")
            nc.sync.dma_start(out=t[:, :], in_=cc_out[:, :])
            nmax = singles.tile([BAGS_PER_CORE, 1], f32, name="nmax")
            nc.vector.reduce_max(out=nmax[:, :], in_=t[:, :], axis=AX.X, negate=True)
            ex = singles.tile([BAGS_PER_CORE, NREL], f32, name="ex")
            nc.scalar.activation(
                out=ex[:, :], in_=t[:, :], func=AF.Exp, bias=nmax[:, :], scale=1.0
            )
            ssum = singles.tile([BAGS_PER_CORE, 1], f32, name="ssum")
            nc.vector.reduce_sum(out=ssum[:, :], in_=ex[:, :], axis=AX.X)
            rcp = singles.tile([BAGS_PER_CORE, 1], f32, name="rcp")
            nc.vector.reciprocal(out=rcp[:, :], in_=ssum[:, :])
            res = singles.tile([BAGS_PER_CORE, NREL], f32, name="res")
            nc.vector.tensor_scalar_mul(res[:, :], ex[:, :], rcp[:, :])
            nc.sync.dma_start(out=out_d[:, :], in_=res[:, :])

    nc.compile()
    return nc


def _get_program():
    global _PROGRAM
    if _PROGRAM is None:
        _PROGRAM = _build_program()
    return _PROGRAM


def _pad_edge(a):
    return np.concatenate([a[:, :1], a, a[:, -1:]], axis=1)


def _col_layout(padded, fill=0):
    """[NS, LP] -> per-core column stream [NBLK, BLK_COLS] (pad cols = fill)."""
    a = padded.reshape(NBLK, SG_PER_BLK, SGS * LP)
    out = np.full((NBLK, SG_PER_BLK, SG_COLS), fill, a.dtype)
    out[:, :, :SGS * LP] = a
    return out.reshape(NBLK, BLK_COLS)


def _token_layout(padded):
    """[NS, LP] int32 -> indirect-gather index layout [128, NBLK*32].

    idx[p, blk*32+t] = column stream value at block col t*128+p."""
    flat = _col_layout(padded, 0).reshape(NBLK, TILES_PER_BLK, 128)
    return flat.transpose(2, 0, 1).reshape(128, NBLK * TILES_PER_BLK)


def prepare_in_maps(**inputs):
    sentences = np.asarray(inputs["sentences"]).astype(np.int32)
    pos1 = np.asarray(inputs["pos1"]).astype(np.int32)
    pos2 = np.asarray(inputs["pos2"]).astype(np.int32)
    masks = np.asarray(inputs["masks"]).astype(np.float32)
    bag_ids = np.asarray(inputs["bag_ids"]).astype(np.int64)
    word_emb = np.asarray(inputs["word_emb"]).astype(np.float32)
    pf1_emb = np.asarray(inputs["pf1_emb"]).astype(np.float32)
    pf2_emb = np.asarray(inputs["pf2_emb"]).astype(np.float32)
    conv_w = np.asarray(inputs["conv_w"]).astype(np.float32)
    conv_b = np.asarray(inputs["conv_b"]).astype(np.float32)
    dense_w = np.asarray(inputs["dense_w"]).astype(np.float32)
    dense_b = np.asarray(inputs["dense_b"]).astype(np.float32)

    # ---- shared (replicated) parameter prep ----
    wemb_q = word_emb.astype(BF16)

    # conv weight pair layout [128ch, fc, pair, view, fw]; chunk-2 views get
    # 54 real channel rows, plus +BMASK at the mask row (54) in the tap-1
    # view only.  Packed into one DRAM tensor: pairs then the (2,2) single.
    def _wview(fc, v):
        f0, fw = FCH[fc]
        c, k = v
        if c == 3:
            c, k = 2, 1  # replica serves chunk2 tap 1
        out = np.zeros((128, 128), np.float32)
        nch = 128 if c < 2 else WD + 2 * PD - 256
        out[:nch, :fw] = conv_w[f0:f0 + fw, c * 128:c * 128 + nch, k].T
        if c == 2 and k == 1:
            out[54, :fw] = BMASK
        return out

    wtp = np.zeros((128, 2, 4, 2, 128), np.float32)
    wts = np.zeros((128, 2, 128), np.float32)
    for fc in range(2):
        for p, (vA, vB) in enumerate(PAIRS):
            wtp[:, fc, p, 0] = _wview(fc, vA)
            wtp[:, fc, p, 1] = _wview(fc, vB)
        wts[:, fc] = _wview(fc, SINGLE)
    wtp_packed = np.zeros((128, 2560), np.float32)
    wtp_packed[:, :2048] = wtp.reshape(128, 2048)
    wtp_packed[:, 2048:2304] = wts.reshape(128, 256)
    wtp_packed = wtp_packed.astype(FP8)

    selb = np.zeros((4, 2, 128), np.float32)
    selb[1, 0, :] = BMASK   # phase j=1 increment rides mask row 1
    selb[2, 1, :] = BMASK   # phase j=2 increment rides mask row 2
    selb = selb.astype(FP8)

    dwt = np.zeros((128, 6 * NREL), np.float32)
    for j in range(3):
        for fc, (f0, fw) in enumerate(FCH):
            dwt[:fw, (j * 2 + fc) * NREL:(j * 2 + fc + 1) * NREL] = \
                dense_w[:, j * NF + f0:j * NF + f0 + fw].T
    dwt = dwt.astype(BF16)

    actb = np.zeros((128, 2), np.float32)
    for fc, (f0, fw) in enumerate(FCH):
        actb[:fw, fc] = conv_b[f0:f0 + fw] - BMASK

    dbias = (dense_b / NCORES).reshape(1, NREL).astype(BF16)
    id16 = np.eye(128, dtype=BF16)

    counts = np.bincount(bag_ids, minlength=NBAGS).astype(np.float32)
    counts = np.maximum(counts, 1.0)

    # ---- per-core prep ----
    in_maps = []
    for r in range(NCORES):
        sl = slice(r * NS, (r + 1) * NS)
        idxw = _token_layout(_pad_edge(sentences[sl])).astype(np.int32)

        m = masks[sl]  # [256, 3, 120]
        md = np.stack([m[:, 0], m[:, 1] - m[:, 0], m[:, 2] - m[:, 1]], axis=1)

        p1p = _pad_edge(pos1[sl])
        p2p = _pad_edge(pos2[sl])
        pfv = np.concatenate([pf1_emb[p1p], pf2_emb[p2p]], axis=2)  # [NS, LP, 10]
        xpf = np.zeros((NBLK, 11, BLK_COLS), np.float32)
        for d in range(2 * PD):
            xpf[:, d, :] = _col_layout(pfv[:, :, d].reshape(NS, LP), 0.0)
        # mask m0 row, center-tap aligned (column t+1 within each sentence)
        mrow = np.zeros((NS, LP), np.float32)
        mrow[:, 1:L + 1] = md[:, 0, :]
        xpf[:, 10, :] = _col_layout(mrow, 0.0)
        xpf = xpf.astype(FP8)

        # rows 1,2 hold the phase increments (m1-m0, m2-m1) in the padded
        # 122-col-per-sentence subgroup layout; selb picks partition j+1
        masksd = np.zeros((NBLK, 4, BLK_COLS), np.float32)
        for j in (1, 2):
            mdpad = np.zeros((NS, LP), np.float32)
            mdpad[:, :L] = md[:, j, :]
            masksd[:, j, :] = _col_layout(mdpad, 0.0)
        masksd = masksd.astype(FP8)

        bags = bag_ids[sl]
        snorm = np.zeros((NS, NBAGS), np.float32)
        snorm[np.arange(NS), bags] = 1.0 / counts[bags]
        snorm = snorm.astype(BF16)

        in_maps.append({
            "wemb": wemb_q,
            "idxw": idxw,
            "xpf": xpf,
            "masksd": masksd,
            "snorm": snorm,
            "wtp": wtp_packed,
            "selb": selb.reshape(4, -1),
            "dwt": dwt,
            "actb": actb,
            "dbias": dbias,
            "id16d": id16,
        })
    return in_maps


def kernel(**inputs):
    in_maps = prepare_in_maps(**inputs)
    nc = _get_program()
    from concourse.bass_utils import run_bass_kernel_spmd

    trace = bool(int(os.environ.get("KERNEL_TRACE", "0")))
    res = run_bass_kernel_spmd(
        nc, in_maps, core_ids=list(range(NCORES)), trace=trace
    )
    global LAST_RESULT
    LAST_RESULT = res
    out = np.concatenate([res.results[r]["out"] for r in range(NCORES)], axis=0)
    return out.astype(np.float32)


if __name__ == "__main__":
    d = np.load("/root/problem/ref_inputs.npz")
    out = kernel(**{k: d[k] for k in d.files})
    print("out", out.shape, out.dtype)
    ref = np.load("/root/problem/ref_out.npy")
    err = np.abs(out - ref).max() / np.abs(ref).max()
    print("Relative error:", err)


# revision 31
# speedup vs baseline: 1.2078x; 1.2078x over previous
"""Trainium2 Bass kernel for the PCNN (piecewise-CNN) bag-classification model.

v2b pipeline (per NeuronCore, data-parallel over sentences, 256/core):
  1. ONE batched indirect-DMA gather per 32-sentence block (4096 fp8 embedding
     rows; 8 gpsimd instructions/core vs 256 -> SWDGE fixed cost ~19us not 280)
  2. PE transposes (fp8) -> channel-major xcq [128, 4, 4112]; chunk 3 is a
     +15-column-shifted replica of chunk 2 (SBUF->SBUF DMA) so its tap views
     are 16-byte aligned for DoubleRow pairing
  3. conv1d(k=3, edge-pad) as 4 fp8 DoubleRow pair-matmuls (256-deep
     contraction, 0.5 cyc/row) + 1 plain fp8 matmul per (subgroup, filter
     chunk); +128*mask piece-0 bias rides the center-tap view's row 54
  4. PCNN piecewise max-pool: two fp8 rank-1 increment matmuls (pieces 1,2);
     DVE reduce_max reads both filter-chunk PSUM banks in one op
  5. ReLU(max-128+conv_b), dense to 53 logits, bag segment-mean matmul
  6. ReduceScatter(add) over 8 cores -> each core softmaxes its 32 bags,
     host concatenates the 8 x [32,53] slices
"""

import os
import sys

for _p in ("/opt/trn_rl_repo",):
    if _p not in sys.path:
        sys.path.insert(0, _p)

import numpy as np
import ml_dtypes

# ---------------- problem constants (hardcoded per spec) ----------------
N = 2048          # total sentences
L = 120           # max sentence length
LP = 122          # edge-padded length
NCORES = 8
NS = N // NCORES  # 256 sentences per core
BLK = 32          # sentences per block
NBLK = NS // BLK  # 8 blocks
SGS = 4           # sentences per matmul subgroup
SG_PER_BLK = BLK // SGS          # 8
SG_COLS = 512                    # padded columns per subgroup (4*122=488 real)
BLK_COLS = SG_PER_BLK * SG_COLS  # 4096
TILES_PER_BLK = BLK_COLS // 128  # 32
NF = 230
NREL = 53
NBAGS = 256
BAGS_PER_CORE = NBAGS // NCORES  # 32
VOCAB = 100000
WD = 300
PD = 5
ELEM = 300        # gathered bf16 row length == table row stride (indirect
                  # gather's index coefficient comes from the src shape)
BMASK = 128.0     # piecewise-pool mask bias (exact in fp8/bf16)
FCH = [(0, 128), (128, 102)]     # filter chunks
CCH = [(0, 128), (128, 128), (256, 44)]  # word-channel transpose chunks
CSTR = 4112       # xcq chunk stride (4096 cols + room for the +15 replica)
RSHIFT = 15
# DoubleRow k-tile pair views (chunk, tap); pair-dim step must be %16 fp8
# bytes, so taps pair across chunks (step 4112) and chunk2 tap1 reads the
# +15-shifted replica in chunk slot 3 (step 4128). (2,2) runs as a plain
# fp8 single. The PCNN mask row (channel 310 = chunk2 partition 54, stored
# center-tap aligned) gets weight +BMASK only in the tap-1 view.
PAIRS = [((0, 0), (1, 0)), ((0, 1), (1, 1)), ((0, 2), (1, 2)),
         ((2, 0), (3, 1))]
SINGLE = (2, 2)

BF16 = ml_dtypes.bfloat16
FP8 = ml_dtypes.float8_e4m3

_PROGRAM = None
LAST_RESULT = None


def _view_off(v):
    """fp8 free-dim offset of a k-tile view within xcq [128, 4, CSTR]."""
    c, k = v
    if c == 3:  # chunk2 replica, shifted by RSHIFT
        return 3 * CSTR + RSHIFT + k
    return c * CSTR + k


def _build_program():
    import concourse.bass as bass
    import concourse.mybir as mybir
    import concourse.tile as tile
    from concourse import bacc

    f32 = mybir.dt.float32
    bf16 = mybir.dt.bfloat16
    fp8 = mybir.dt.float8e4
    i32 = mybir.dt.int32
    AF = mybir.ActivationFunctionType
    AX = mybir.AxisListType
    PM = mybir.MatmulPerfMode

    nc = bacc.Bacc(
        "TRN2",
        target_bir_lowering=False,
        debug=False,
        num_devices=NCORES,
    )

    # ------------- external I/O -------------
    wemb = nc.dram_tensor("wemb", [VOCAB, ELEM], bf16, kind="ExternalInput").ap()
    idxw_d = nc.dram_tensor("idxw", [128, NBLK * TILES_PER_BLK], i32,
                            kind="ExternalInput").ap()
    xpf_d = nc.dram_tensor("xpf", [NBLK, 11, BLK_COLS], fp8, kind="ExternalInput").ap()
    masks_d = nc.dram_tensor("masksd", [NBLK, 4, BLK_COLS], fp8,
                             kind="ExternalInput").ap()
    snorm_d = nc.dram_tensor("snorm", [NS, NBAGS], bf16, kind="ExternalInput").ap()
    wtp_d = nc.dram_tensor("wtp", [128, 2560], fp8, kind="ExternalInput").ap()
    selb_d = nc.dram_tensor("selb", [4, 2 * 128], fp8, kind="ExternalInput").ap()
    dwt_d = nc.dram_tensor("dwt", [128, 6 * NREL], bf16, kind="ExternalInput").ap()
    actb_d = nc.dram_tensor("actb", [128, 2], f32, kind="ExternalInput").ap()
    dbias_d = nc.dram_tensor("dbias", [1, NREL], bf16, kind="ExternalInput").ap()
    id16_d = nc.dram_tensor("id16d", [128, 128], bf16, kind="ExternalInput").ap()
    out_d = nc.dram_tensor("out", [BAGS_PER_CORE, NREL], f32, kind="ExternalOutput").ap()
    debug = bool(int(os.environ.get("KERNEL_DEBUG", "0")))
    if debug:
        dbg_xcq = nc.dram_tensor("dbg_xcq", [128, 4, CSTR], fp8,
                                 kind="ExternalOutput").ap()
        dbg_pooled = nc.dram_tensor("dbg_pooled", [128, 2, 3, NS], f32,
                                    kind="ExternalOutput").ap()
        dbg_bag = nc.dram_tensor("dbg_bag", [NBAGS, NREL], f32,
                                 kind="ExternalOutput").ap()

    with tile.TileContext(nc) as tc:
        import contextlib

        ctx = contextlib.ExitStack()
        with ctx:
            singles = ctx.enter_context(tc.tile_pool(name="singles", bufs=1))

            # persistent tiles
            wtv_sb = singles.tile([128, 2, 9, 128], fp8, name="wtv")
            selb_sb = singles.tile([4, 2, 128], fp8, name="selb")
            snorm_sb = [singles.tile([128, NBAGS], bf16, name=f"sn{c}") for c in range(2)]
            idxw_sb = singles.tile([128, NBLK * TILES_PER_BLK], i32, name="idx")
            dwt_sb = singles.tile([128, 6 * NREL], bf16)
            actb_sb = singles.tile([128, 2], f32)
            dbias_sb = singles.tile([1, NREL], bf16)
            id16 = singles.tile([128, 128], bf16, name="id16")
            ones_sb = singles.tile([1, 128], bf16)
            pooled = singles.tile([128, 2, 3, NS], f32, name="pool")

            nc.sync.dma_start(out=wtv_sb[:, :, :, :], in_=wtp_d[:, 0:2304])
            nc.sync.dma_start(out=selb_sb[:, :, :], in_=selb_d[:, :])
            for c in range(2):
                nc.sync.dma_start(out=snorm_sb[c][:, :], in_=snorm_d[c * 128:(c + 1) * 128, :])
            nc.sync.dma_start(out=idxw_sb[:, :], in_=idxw_d[:, :])
            nc.sync.dma_start(out=dwt_sb[:, :], in_=dwt_d[:, :])
            nc.sync.dma_start(out=actb_sb[:, :], in_=actb_d[:, :])
            nc.sync.dma_start(out=dbias_sb[:, :], in_=dbias_d[:, :])
            nc.sync.dma_start(out=id16[:, :], in_=id16_d[:, :])
            nc.vector.memset(ones_sb[:, :], 1.0)

            xg_pool = ctx.enter_context(tc.tile_pool(name="xg", bufs=2))
            xcq_pool = ctx.enter_context(tc.tile_pool(name="xcq", bufs=2))
            mask_pool = ctx.enter_context(tc.tile_pool(name="mask", bufs=2))
            tp_psum = ctx.enter_context(tc.tile_pool(name="tp", bufs=2, space="PSUM"))
            cv_psum = ctx.enter_context(tc.tile_pool(name="cv", bufs=3, space="PSUM"))

            NCV = SGS * LP  # 488 contiguous conv output columns per subgroup

            def emit_conv(blk, sg, xcq, ps):
                # fw padded to 128 (zero weight cols) so every PSUM row in both
                # banks is written; out-partition count doesn't affect PE time.
                # chunk2 rows 55:128 are never written, so its views contract
                # only K=55 partitions (same math: weights were 0).
                for fc in range(2):
                    out_ap = ps[0:128, fc, 0:NCV]
                    for v in range(9):
                        c, k = v // 3, v % 3
                        kp = 55 if c == 2 else 128
                        rb = xcq[0:kp, 0, 0:1]
                        off = c * CSTR + k + sg * SG_COLS
                        rhs = bass.AP(tensor=rb.tensor, offset=rb.offset + off,
                                      ap=[rb.ap[0], [1, NCV]])
                        nc.tensor.matmul(
                            out=out_ap,
                            lhsT=wtv_sb[0:kp, fc, v, 0:128],
                            rhs=rhs,
                            start=(v == 0),
                            stop=False,
                            skip_group_check=True,
                        )

            def emit_jphases(blk, sg, mask_sb, ps):
                s0 = blk * BLK + sg * SGS
                for j in range(3):
                    pb0 = ps[0:128, 0, 0:1]
                    rin = bass.AP(
                        tensor=pb0.tensor, offset=pb0.offset,
                        ap=[pb0.ap[0], [SG_COLS, 2], [LP, SGS], [1, L]],
                    )
                    pb = pooled[0:128, 0, j, s0:s0 + SGS]
                    rout = bass.AP(tensor=pb.tensor, offset=pb.offset,
                                   ap=[pb.ap[0], [3 * NS, 2], [1, SGS]])
                    nc.vector.reduce_max(out=rout, in_=rin, axis=AX.X)
                    if j < 2:
                        for fc in range(2):
                            nc.tensor.matmul(
                                out=ps[0:128, fc, 0:NCV],
                                lhsT=selb_sb[0:4, j, 0:128],
                                rhs=mask_sb[0:4, sg * SG_COLS:sg * SG_COLS + NCV],
                                start=False,
                                stop=(j == 1),
                                skip_group_check=True,
                            )

            pending = []
            for blk in range(NBLK):
                # ---- batched gather (token-major, fp8) ----
                xg = xg_pool.tile([128, TILES_PER_BLK, ELEM], bf16, tag="xg")
                for t in range(TILES_PER_BLK):
                    col = blk * TILES_PER_BLK + t
                    nc.gpsimd.indirect_dma_start(
                        out=xg[:, t, 0:WD],
                        out_offset=None,
                        in_=wemb[:, 0:WD],
                        in_offset=bass.IndirectOffsetOnAxis(
                            ap=idxw_sb[:, col:col + 1], axis=0),
                    )
                mask_sb = mask_pool.tile([4, BLK_COLS], fp8, tag="mask")
                nc.sync.dma_start(out=mask_sb[:, :], in_=masks_d[blk, :, :])

                # ---- transpose to channel-major (fp8) ----
                xcq = xcq_pool.tile([128, 4, CSTR], fp8, tag="xcq")
                nc.sync.dma_start(out=xcq[44:55, 2, 0:BLK_COLS], in_=xpf_d[blk, :, :])
                for grp in range(4):  # 8 token-tiles per group
                    for cc, (c0, pw) in enumerate(CCH):
                        tpA = tp_psum.tile([128, 4, 128], bf16, tag="tp", name=f"tpA{cc}")
                        tpB = tp_psum.tile([128, 4, 128], bf16, tag="tp", name=f"tpB{cc}")
                        for t in range(8):
                            ti = grp * 8 + t
                            tgt = tpA if t % 2 == 0 else tpB
                            nc.tensor.transpose(
                                out=tgt[0:pw, t // 2, :],
                                in_=xg[:, ti, c0:c0 + pw],
                                identity=id16[:, :],
                            )
                        for half, tp in ((0, tpA), (1, tpB)):
                            cb = xcq[0:pw, cc, grp * 1024 + half * 128:
                                     grp * 1024 + half * 128 + 1]
                            dst = bass.AP(
                                tensor=cb.tensor, offset=cb.offset,
                                ap=[cb.ap[0], [256, 4], [1, 128]],
                            )
                            nc.scalar.copy(out=dst, in_=tp[0:pw, :, :])

                if debug and blk == 0:
                    nc.sync.dma_start(out=dbg_xcq[:, :, :], in_=xcq[:, :, :])

                for sg in range(SG_PER_BLK):
                    ps = cv_psum.tile([128, 2, SG_COLS], f32, tag="cv",
                                      name=f"cv{blk}_{sg}")
                    emit_conv(blk, sg, xcq, ps)
                    if pending:
                        emit_jphases(*pending.pop(0))
                    pending.append((blk, sg, mask_sb, ps))

            while pending:
                emit_jphases(*pending.pop(0))

            # ---------------- tail ----------------
            if debug:
                nc.sync.dma_start(out=dbg_pooled[:, :, :, :], in_=pooled[:, :, :, :])
            pr = [singles.tile([128, 3, NS], bf16, name=f"pr{c}") for c in range(2)]
            for fc in range(2):
                nc.scalar.activation(
                    out=pr[fc][:, :, :],
                    in_=pooled[:, fc, :, :],
                    func=AF.Relu,
                    bias=actb_sb[:, fc:fc + 1],
                    scale=1.0,
                )

            # dense: logitsT [53, 256] = sum_{j,fc} dwt[(j,fc)].T @ pr
            lg_ps = cv_psum.tile([NREL, NS], f32, tag="cv", name="lgps")
            nmm = 0
            for j in range(3):
                for fc, (f0, fw) in enumerate(FCH):
                    nc.tensor.matmul(
                        out=lg_ps[:, :],
                        lhsT=dwt_sb[0:fw, (j * 2 + fc) * NREL:(j * 2 + fc + 1) * NREL],
                        rhs=pr[fc][0:fw, j, :],
                        start=(nmm == 0),
                        stop=(nmm == 5),
                    )
                    nmm += 1
            lg_sb = singles.tile([NREL, NS], bf16)
            nc.vector.tensor_copy(out=lg_sb[:, :], in_=lg_ps[:, :])

            # transpose logits -> per-sentence rows [256, 53]
            ls = [singles.tile([128, NREL], bf16, name=f"ls{c}") for c in range(2)]
            for sc in range(2):
                ltp = cv_psum.tile([128, NREL], bf16, tag="cv", name="ltp")
                nc.tensor.transpose(
                    out=ltp[0:128, 0:NREL],
                    in_=lg_sb[:, sc * 128:(sc + 1) * 128],
                    identity=id16[0:NREL, 0:NREL],
                )
                nc.vector.tensor_copy(out=ls[sc][:, :], in_=ltp[0:128, 0:NREL])

            # bag aggregation (+ dense bias/8), full 256 bags of partials
            cc_dram = ctx.enter_context(tc.tile_pool(name="ccd", bufs=1, space="DRAM"))
            cc_in = cc_dram.tile([NBAGS, NREL], f32)
            cc_out = cc_dram.tile([BAGS_PER_CORE, NREL], f32)
            for bc in range(2):
                bg = cv_psum.tile([128, NREL], f32, tag="cv", name="bg")
                for sc in range(2):
                    nc.tensor.matmul(
                        out=bg[:, :],
                        lhsT=snorm_sb[sc][:, bc * 128:(bc + 1) * 128],
                        rhs=ls[sc][:, :],
                        start=(sc == 0),
                        stop=False,
                    )
                nc.tensor.matmul(
                    out=bg[:, :],
                    lhsT=ones_sb[0:1, 0:128],
                    rhs=dbias_sb[0:1, :],
                    start=False,
                    stop=True,
                )
                bg_sb = singles.tile([128, NREL], f32, name=f"bgs{bc}")
                nc.vector.tensor_copy(out=bg_sb[:, :], in_=bg[:, :])
                nc.sync.dma_start(out=cc_in[bc * 128:(bc + 1) * 128, :], in_=bg_sb[:, :])

            if debug:
                nc.sync.dma_start(out=dbg_bag[:, :], in_=cc_in[:, :])
            nc.gpsimd.collective_compute(
                "ReduceScatter",
                mybir.AluOpType.add,
                replica_groups=[list(range(NCORES))],
                ins=[cc_in.opt()],
                outs=[cc_out.opt()],
            )

            # softmax over the 53 relations for this core's 32 bags
            t = singles.tile([BAGS_PER_CORE, NREL], f32, name="sm")
            nc.sync.dma_start(out=t[:, :], in_=cc_out[:, :])
            nmax = singles.tile([BAGS_PER_CORE, 1], f32, name="nmax")
            nc.vector.reduce_max(out=nmax[:, :], in_=t[:, :], axis=AX.X, negate=True)
            ex = singles.tile([BAGS_PER_CORE, NREL], f32, name="ex")
            nc.scalar.activation(
                out=ex[:, :], in_=t[:, :], func=AF.Exp, bias=nmax[:, :], scale=1.0
            )
            ssum = singles.tile([BAGS_PER_CORE, 1], f32, name="ssum")
            nc.vector.reduce_sum(out=ssum[:, :], in_=ex[:, :], axis=AX.X)
            rcp = singles.tile([BAGS_PER_CORE, 1], f32, name="rcp")
            nc.vector.reciprocal(out=rcp[:, :], in_=ssum[:, :])
            res = singles.tile([BAGS_PER_CORE, NREL], f32, name="res")
            nc.vector.tensor_scalar_mul(res[:, :], ex[:, :], rcp[:, :])
            nc.sync.dma_start(out=out_d[:, :], in_=res[:, :])

    nc.compile()
    return nc


def _get_program():
    global _PROGRAM
    if _PROGRAM is None:
        _PROGRAM = _build_program()
    return _PROGRAM


def _pad_edge(a):
    return np.concatenate([a[:, :1], a, a[:, -1:]], axis=1)


def _col_layout(padded, fill=0):
    """[NS, LP] -> per-core column stream [NBLK, BLK_COLS] (pad cols = fill)."""
    a = padded.reshape(NBLK, SG_PER_BLK, SGS * LP)
    out = np.full((NBLK, SG_PER_BLK, SG_COLS), fill, a.dtype)
    out[:, :, :SGS * LP] = a
    return out.reshape(NBLK, BLK_COLS)


def _token_layout(padded):
    """[NS, LP] int32 -> indirect-gather index layout [128, NBLK*32].

    idx[p, blk*32+t] = column stream value at block col t*128+p."""
    flat = _col_layout(padded, 0).reshape(NBLK, TILES_PER_BLK, 128)
    return flat.transpose(2, 0, 1).reshape(128, NBLK * TILES_PER_BLK)


def prepare_in_maps(**inputs):
    sentences = np.asarray(inputs["sentences"]).astype(np.int32)
    pos1 = np.asarray(inputs["pos1"]).astype(np.int32)
    pos2 = np.asarray(inputs["pos2"]).astype(np.int32)
    masks = np.asarray(inputs["masks"]).astype(np.float32)
    bag_ids = np.asarray(inputs["bag_ids"]).astype(np.int64)
    word_emb = np.asarray(inputs["word_emb"]).astype(np.float32)
    pf1_emb = np.asarray(inputs["pf1_emb"]).astype(np.float32)
    pf2_emb = np.asarray(inputs["pf2_emb"]).astype(np.float32)
    conv_w = np.asarray(inputs["conv_w"]).astype(np.float32)
    conv_b = np.asarray(inputs["conv_b"]).astype(np.float32)
    dense_w = np.asarray(inputs["dense_w"]).astype(np.float32)
    dense_b = np.asarray(inputs["dense_b"]).astype(np.float32)

    # ---- shared (replicated) parameter prep ----
    wemb_q = word_emb.astype(BF16)

    # conv weight pair layout [128ch, fc, pair, view, fw]; chunk-2 views get
    # 54 real channel rows, plus +BMASK at the mask row (54) in the tap-1
    # view only.  Packed into one DRAM tensor: pairs then the (2,2) single.
    def _wview(fc, v):
        f0, fw = FCH[fc]
        c, k = v
        if c == 3:
            c, k = 2, 1  # replica serves chunk2 tap 1
        out = np.zeros((128, 128), np.float32)
        nch = 128 if c < 2 else WD + 2 * PD - 256
        out[:nch, :fw] = conv_w[f0:f0 + fw, c * 128:c * 128 + nch, k].T
        if c == 2 and k == 1:
            out[54, :fw] = BMASK
        return out

    wtv = np.zeros((128, 2, 9, 128), np.float32)
    for fc in range(2):
        for v in range(9):
            wtv[:, fc, v] = _wview(fc, (v // 3, v % 3))
    wtp_packed = np.zeros((128, 2560), np.float32)
    wtp_packed[:, :2304] = wtv.reshape(128, 2304)
    wtp_packed = wtp_packed.astype(FP8)

    selb = np.zeros((4, 2, 128), np.float32)
    selb[1, 0, :] = BMASK   # phase j=1 increment rides mask row 1
    selb[2, 1, :] = BMASK   # phase j=2 increment rides mask row 2
    selb = selb.astype(FP8)

    dwt = np.zeros((128, 6 * NREL), np.float32)
    for j in range(3):
        for fc, (f0, fw) in enumerate(FCH):
            dwt[:fw, (j * 2 + fc) * NREL:(j * 2 + fc + 1) * NREL] = \
                dense_w[:, j * NF + f0:j * NF + f0 + fw].T
    dwt = dwt.astype(BF16)

    actb = np.zeros((128, 2), np.float32)
    for fc, (f0, fw) in enumerate(FCH):
        actb[:fw, fc] = conv_b[f0:f0 + fw] - BMASK

    dbias = (dense_b / NCORES).reshape(1, NREL).astype(BF16)
    id16 = np.eye(128, dtype=BF16)

    counts = np.bincount(bag_ids, minlength=NBAGS).astype(np.float32)
    counts = np.maximum(counts, 1.0)

    # ---- per-core prep ----
    in_maps = []
    for r in range(NCORES):
        sl = slice(r * NS, (r + 1) * NS)
        idxw = _token_layout(_pad_edge(sentences[sl])).astype(np.int32)

        m = masks[sl]  # [256, 3, 120]
        md = np.stack([m[:, 0], m[:, 1] - m[:, 0], m[:, 2] - m[:, 1]], axis=1)

        p1p = _pad_edge(pos1[sl])
        p2p = _pad_edge(pos2[sl])
        pfv = np.concatenate([pf1_emb[p1p], pf2_emb[p2p]], axis=2)  # [NS, LP, 10]
        xpf = np.zeros((NBLK, 11, BLK_COLS), np.float32)
        for d in range(2 * PD):
            xpf[:, d, :] = _col_layout(pfv[:, :, d].reshape(NS, LP), 0.0)
        # mask m0 row, center-tap aligned (column t+1 within each sentence)
        mrow = np.zeros((NS, LP), np.float32)
        mrow[:, 1:L + 1] = md[:, 0, :]
        xpf[:, 10, :] = _col_layout(mrow, 0.0)
        xpf = xpf.astype(FP8)

        # rows 1,2 hold the phase increments (m1-m0, m2-m1) in the padded
        # 122-col-per-sentence subgroup layout; selb picks partition j+1
        masksd = np.zeros((NBLK, 4, BLK_COLS), np.float32)
        for j in (1, 2):
            mdpad = np.zeros((NS, LP), np.float32)
            mdpad[:, :L] = md[:, j, :]
            masksd[:, j, :] = _col_layout(mdpad, 0.0)
        masksd = masksd.astype(FP8)

        bags = bag_ids[sl]
        snorm = np.zeros((NS, NBAGS), np.float32)
        snorm[np.arange(NS), bags] = 1.0 / counts[bags]
        snorm = snorm.astype(BF16)

        in_maps.append({
            "wemb": wemb_q,
            "idxw": idxw,
            "xpf": xpf,
            "masksd": masksd,
            "snorm": snorm,
            "wtp": wtp_packed,
            "selb": selb.reshape(4, -1),
            "dwt": dwt,
            "actb": actb,
            "dbias": dbias,
            "id16d": id16,
        })
    return in_maps


def kernel(**inputs):
    in_maps = prepare_in_maps(**inputs)
    nc = _get_program()
    from concourse.bass_utils import run_bass_kernel_spmd

    trace = bool(int(os.environ.get("KERNEL_TRACE", "0")))
    res = run_bass_kernel_spmd(
        nc, in_maps, core_ids=list(range(NCORES)), trace=trace
    )
    global LAST_RESULT
    LAST_RESULT = res
    out = np.concatenate([res.results[r]["out"] for r in range(NCORES)], axis=0)
    return out.astype(np.float32)


if __name__ == "__main__":
    d = np.load("/root/problem/ref_inputs.npz")
    out = kernel(**{k: d[k] for k in d.files})
    print("out", out.shape, out.dtype)
    ref = np.load("/root/problem/ref_out.npy")
    err = np.abs(out - ref).max() / np.abs(ref).max()
    print("Relative error:", err)


# revision 33
# speedup vs baseline: 1.2343x; 1.0220x over previous
"""Trainium2 Bass kernel for the PCNN (piecewise-CNN) bag-classification model.

v2b pipeline (per NeuronCore, data-parallel over sentences, 256/core):
  1. ONE batched indirect-DMA gather per 32-sentence block (4096 fp8 embedding
     rows; 8 gpsimd instructions/core vs 256 -> SWDGE fixed cost ~19us not 280)
  2. PE transposes (fp8) -> channel-major xcq [128, 4, 4112]; chunk 3 is a
     +15-column-shifted replica of chunk 2 (SBUF->SBUF DMA) so its tap views
     are 16-byte aligned for DoubleRow pairing
  3. conv1d(k=3, edge-pad) as 4 fp8 DoubleRow pair-matmuls (256-deep
     contraction, 0.5 cyc/row) + 1 plain fp8 matmul per (subgroup, filter
     chunk); +128*mask piece-0 bias rides the center-tap view's row 54
  4. PCNN piecewise max-pool: two fp8 rank-1 increment matmuls (pieces 1,2);
     DVE reduce_max reads both filter-chunk PSUM banks in one op
  5. ReLU(max-128+conv_b), dense to 53 logits, bag segment-mean matmul
  6. ReduceScatter(add) over 8 cores -> each core softmaxes its 32 bags,
     host concatenates the 8 x [32,53] slices
"""

import os
import sys

for _p in ("/opt/trn_rl_repo",):
    if _p not in sys.path:
        sys.path.insert(0, _p)

import numpy as np
import ml_dtypes

# ---------------- problem constants (hardcoded per spec) ----------------
N = 2048          # total sentences
L = 120           # max sentence length
LP = 122          # edge-padded length
NCORES = 8
NS = N // NCORES  # 256 sentences per core
BLK = 32          # sentences per block
NBLK = NS // BLK  # 8 blocks
SGS = 4           # sentences per matmul subgroup
SG_PER_BLK = BLK // SGS          # 8
SG_COLS = 512                    # padded columns per subgroup (4*122=488 real)
BLK_COLS = SG_PER_BLK * SG_COLS  # 4096
TILES_PER_BLK = BLK_COLS // 128  # 32
NF = 230
NREL = 53
NBAGS = 256
BAGS_PER_CORE = NBAGS // NCORES  # 32
VOCAB = 100000
WD = 300
PD = 5
ELEM = 300        # gathered bf16 row length == table row stride (indirect
                  # gather's index coefficient comes from the src shape)
BMASK = 128.0     # piecewise-pool mask bias (exact in fp8/bf16)
FCH = [(0, 128), (128, 102)]     # filter chunks
CCH = [(0, 128), (128, 128), (256, 44)]  # word-channel transpose chunks
CSTR = 4112       # xcq chunk stride (4096 cols + room for the +15 replica)
RSHIFT = 15
# DoubleRow k-tile pair views (chunk, tap); pair-dim step must be %16 fp8
# bytes, so taps pair across chunks (step 4112) and chunk2 tap1 reads the
# +15-shifted replica in chunk slot 3 (step 4128). (2,2) runs as a plain
# fp8 single. The PCNN mask row (channel 310 = chunk2 partition 54, stored
# center-tap aligned) gets weight +BMASK only in the tap-1 view.
PAIRS = [((0, 0), (1, 0)), ((0, 1), (1, 1)), ((0, 2), (1, 2)),
         ((2, 0), (3, 1))]
SINGLE = (2, 2)

BF16 = ml_dtypes.bfloat16
FP8 = ml_dtypes.float8_e4m3

_PROGRAM = None
LAST_RESULT = None


def _view_off(v):
    """fp8 free-dim offset of a k-tile view within xcq [128, 4, CSTR]."""
    c, k = v
    if c == 3:  # chunk2 replica, shifted by RSHIFT
        return 3 * CSTR + RSHIFT + k
    return c * CSTR + k


def _build_program():
    import concourse.bass as bass
    import concourse.mybir as mybir
    import concourse.tile as tile
    from concourse import bacc

    f32 = mybir.dt.float32
    bf16 = mybir.dt.bfloat16
    fp8 = mybir.dt.float8e4
    i32 = mybir.dt.int32
    AF = mybir.ActivationFunctionType
    AX = mybir.AxisListType
    PM = mybir.MatmulPerfMode

    nc = bacc.Bacc(
        "TRN2",
        target_bir_lowering=False,
        debug=False,
        num_devices=NCORES,
    )

    # ------------- external I/O -------------
    wemb = nc.dram_tensor("wemb", [VOCAB, ELEM], bf16, kind="ExternalInput").ap()
    idxw_d = nc.dram_tensor("idxw", [128, NBLK * TILES_PER_BLK], i32,
                            kind="ExternalInput").ap()
    xpf_d = nc.dram_tensor("xpf", [NBLK, 11, BLK_COLS], fp8, kind="ExternalInput").ap()
    masks_d = nc.dram_tensor("masksd", [NBLK, 4, BLK_COLS], fp8,
                             kind="ExternalInput").ap()
    snorm_d = nc.dram_tensor("snorm", [NS, NBAGS], bf16, kind="ExternalInput").ap()
    wtp_d = nc.dram_tensor("wtp", [128, 2560], fp8, kind="ExternalInput").ap()
    selb_d = nc.dram_tensor("selb", [4, 2 * 128], fp8, kind="ExternalInput").ap()
    dwt_d = nc.dram_tensor("dwt", [128, 6 * NREL], bf16, kind="ExternalInput").ap()
    actb_d = nc.dram_tensor("actb", [128, 2], f32, kind="ExternalInput").ap()
    dbias_d = nc.dram_tensor("dbias", [1, NREL], bf16, kind="ExternalInput").ap()
    id16_d = nc.dram_tensor("id16d", [128, 128], bf16, kind="ExternalInput").ap()
    out_d = nc.dram_tensor("out", [NBAGS, NREL], f32, kind="ExternalOutput").ap()
    debug = bool(int(os.environ.get("KERNEL_DEBUG", "0")))
    if debug:
        dbg_xcq = nc.dram_tensor("dbg_xcq", [128, 4, CSTR], fp8,
                                 kind="ExternalOutput").ap()
        dbg_pooled = nc.dram_tensor("dbg_pooled", [128, 2, 3, NS], f32,
                                    kind="ExternalOutput").ap()
        dbg_bag = nc.dram_tensor("dbg_bag", [NBAGS, NREL], f32,
                                 kind="ExternalOutput").ap()

    with tile.TileContext(nc) as tc:
        import contextlib

        ctx = contextlib.ExitStack()
        with ctx:
            singles = ctx.enter_context(tc.tile_pool(name="singles", bufs=1))

            # persistent tiles
            wtv_sb = singles.tile([128, 2, 9, 128], fp8, name="wtv")
            selb_sb = singles.tile([4, 2, 128], fp8, name="selb")
            snorm_sb = [singles.tile([128, NBAGS], bf16, name=f"sn{c}") for c in range(2)]
            idxw_sb = singles.tile([128, NBLK * TILES_PER_BLK], i32, name="idx")
            dwt_sb = singles.tile([128, 6 * NREL], bf16)
            actb_sb = singles.tile([128, 2], f32)
            dbias_sb = singles.tile([1, NREL], bf16)
            id16 = singles.tile([128, 128], bf16, name="id16")
            ones_sb = singles.tile([1, 128], bf16)
            pooled = singles.tile([128, 2, 3, NS], f32, name="pool")

            nc.sync.dma_start(out=wtv_sb[:, :, :, :], in_=wtp_d[:, 0:2304])
            nc.sync.dma_start(out=selb_sb[:, :, :], in_=selb_d[:, :])
            for c in range(2):
                nc.sync.dma_start(out=snorm_sb[c][:, :], in_=snorm_d[c * 128:(c + 1) * 128, :])
            nc.sync.dma_start(out=idxw_sb[:, :], in_=idxw_d[:, :])
            nc.sync.dma_start(out=dwt_sb[:, :], in_=dwt_d[:, :])
            nc.sync.dma_start(out=actb_sb[:, :], in_=actb_d[:, :])
            nc.sync.dma_start(out=dbias_sb[:, :], in_=dbias_d[:, :])
            nc.sync.dma_start(out=id16[:, :], in_=id16_d[:, :])
            nc.vector.memset(ones_sb[:, :], 1.0)

            xg_pool = ctx.enter_context(tc.tile_pool(name="xg", bufs=3))
            xcq_pool = ctx.enter_context(tc.tile_pool(name="xcq", bufs=2))
            mask_pool = ctx.enter_context(tc.tile_pool(name="mask", bufs=2))
            tp_psum = ctx.enter_context(tc.tile_pool(name="tp", bufs=2, space="PSUM"))
            cv_psum = ctx.enter_context(tc.tile_pool(name="cv", bufs=3, space="PSUM"))

            NCV = SGS * LP  # 488 contiguous conv output columns per subgroup

            def emit_conv(blk, sg, xcq, ps):
                # fw padded to 128 (zero weight cols) so every PSUM row in both
                # banks is written; out-partition count doesn't affect PE time.
                # chunk2 rows 55:128 are never written, so its views contract
                # only K=55 partitions (same math: weights were 0).
                for fc in range(2):
                    out_ap = ps[0:128, fc, 0:NCV]
                    for v in range(9):
                        c, k = v // 3, v % 3
                        kp = 55 if c == 2 else 128
                        rb = xcq[0:kp, 0, 0:1]
                        off = c * CSTR + k + sg * SG_COLS
                        rhs = bass.AP(tensor=rb.tensor, offset=rb.offset + off,
                                      ap=[rb.ap[0], [1, NCV]])
                        nc.tensor.matmul(
                            out=out_ap,
                            lhsT=wtv_sb[0:kp, fc, v, 0:128],
                            rhs=rhs,
                            start=(v == 0),
                            stop=False,
                            skip_group_check=True,
                        )

            def emit_jphases(blk, sg, mask_sb, ps):
                s0 = blk * BLK + sg * SGS
                for j in range(3):
                    pb0 = ps[0:128, 0, 0:1]
                    rin = bass.AP(
                        tensor=pb0.tensor, offset=pb0.offset,
                        ap=[pb0.ap[0], [SG_COLS, 2], [LP, SGS], [1, L]],
                    )
                    pb = pooled[0:128, 0, j, s0:s0 + SGS]
                    rout = bass.AP(tensor=pb.tensor, offset=pb.offset,
                                   ap=[pb.ap[0], [3 * NS, 2], [1, SGS]])
                    nc.vector.reduce_max(out=rout, in_=rin, axis=AX.X)
                    if j < 2:
                        for fc in range(2):
                            nc.tensor.matmul(
                                out=ps[0:128, fc, 0:NCV],
                                lhsT=selb_sb[0:4, j, 0:128],
                                rhs=mask_sb[0:4, sg * SG_COLS:sg * SG_COLS + NCV],
                                start=False,
                                stop=(j == 1),
                                skip_group_check=True,
                            )

            pending = []
            for blk in range(NBLK):
                # ---- batched gather (token-major, fp8) ----
                xg = xg_pool.tile([128, TILES_PER_BLK, ELEM], bf16, tag="xg")
                for t in range(TILES_PER_BLK):
                    col = blk * TILES_PER_BLK + t
                    nc.gpsimd.indirect_dma_start(
                        out=xg[:, t, 0:WD],
                        out_offset=None,
                        in_=wemb[:, 0:WD],
                        in_offset=bass.IndirectOffsetOnAxis(
                            ap=idxw_sb[:, col:col + 1], axis=0),
                    )
                mask_sb = mask_pool.tile([4, BLK_COLS], fp8, tag="mask")
                nc.sync.dma_start(out=mask_sb[:, :], in_=masks_d[blk, :, :])

                # ---- transpose to channel-major (fp8) ----
                xcq = xcq_pool.tile([128, 4, CSTR], fp8, tag="xcq")
                nc.sync.dma_start(out=xcq[44:55, 2, 0:BLK_COLS], in_=xpf_d[blk, :, :])
                for grp in range(4):  # 8 token-tiles per group
                    for cc, (c0, pw) in enumerate(CCH):
                        tpA = tp_psum.tile([128, 4, 128], bf16, tag="tp", name=f"tpA{cc}")
                        tpB = tp_psum.tile([128, 4, 128], bf16, tag="tp", name=f"tpB{cc}")
                        for t in range(8):
                            ti = grp * 8 + t
                            tgt = tpA if t % 2 == 0 else tpB
                            nc.tensor.transpose(
                                out=tgt[0:pw, t // 2, :],
                                in_=xg[:, ti, c0:c0 + pw],
                                identity=id16[:, :],
                            )
                        for half, tp in ((0, tpA), (1, tpB)):
                            cb = xcq[0:pw, cc, grp * 1024 + half * 128:
                                     grp * 1024 + half * 128 + 1]
                            dst = bass.AP(
                                tensor=cb.tensor, offset=cb.offset,
                                ap=[cb.ap[0], [256, 4], [1, 128]],
                            )
                            nc.scalar.copy(out=dst, in_=tp[0:pw, :, :])

                if debug and blk == 0:
                    nc.sync.dma_start(out=dbg_xcq[:, :, :], in_=xcq[:, :, :])

                for sg in range(SG_PER_BLK):
                    ps = cv_psum.tile([128, 2, SG_COLS], f32, tag="cv",
                                      name=f"cv{blk}_{sg}")
                    emit_conv(blk, sg, xcq, ps)
                    if pending:
                        emit_jphases(*pending.pop(0))
                    pending.append((blk, sg, mask_sb, ps))

            while pending:
                emit_jphases(*pending.pop(0))

            # ---------------- tail ----------------
            if debug:
                nc.sync.dma_start(out=dbg_pooled[:, :, :, :], in_=pooled[:, :, :, :])
            pr = [singles.tile([128, 3, NS], bf16, name=f"pr{c}") for c in range(2)]
            for fc in range(2):
                nc.scalar.activation(
                    out=pr[fc][:, :, :],
                    in_=pooled[:, fc, :, :],
                    func=AF.Relu,
                    bias=actb_sb[:, fc:fc + 1],
                    scale=1.0,
                )

            # dense: logitsT [53, 256] = sum_{j,fc} dwt[(j,fc)].T @ pr
            lg_ps = cv_psum.tile([NREL, NS], f32, tag="cv", name="lgps")
            nmm = 0
            for j in range(3):
                for fc, (f0, fw) in enumerate(FCH):
                    nc.tensor.matmul(
                        out=lg_ps[:, :],
                        lhsT=dwt_sb[0:fw, (j * 2 + fc) * NREL:(j * 2 + fc + 1) * NREL],
                        rhs=pr[fc][0:fw, j, :],
                        start=(nmm == 0),
                        stop=(nmm == 5),
                    )
                    nmm += 1
            lg_sb = singles.tile([NREL, NS], bf16)
            nc.vector.tensor_copy(out=lg_sb[:, :], in_=lg_ps[:, :])

            # transpose logits -> per-sentence rows [256, 53]
            ls = [singles.tile([128, NREL], bf16, name=f"ls{c}") for c in range(2)]
            for sc in range(2):
                ltp = cv_psum.tile([128, NREL], bf16, tag="cv", name="ltp")
                nc.tensor.transpose(
                    out=ltp[0:128, 0:NREL],
                    in_=lg_sb[:, sc * 128:(sc + 1) * 128],
                    identity=id16[0:NREL, 0:NREL],
                )
                nc.vector.tensor_copy(out=ls[sc][:, :], in_=ltp[0:128, 0:NREL])

            # bag aggregation (+ dense bias/8), full 256 bags of partials
            cc_dram = ctx.enter_context(tc.tile_pool(name="ccd", bufs=1, space="DRAM"))
            cc_in = cc_dram.tile([NBAGS, NREL], f32)
            cc_out = cc_dram.tile([NBAGS, NREL], f32)
            for bc in range(2):
                bg = cv_psum.tile([128, NREL], f32, tag="cv", name="bg")
                for sc in range(2):
                    nc.tensor.matmul(
                        out=bg[:, :],
                        lhsT=snorm_sb[sc][:, bc * 128:(bc + 1) * 128],
                        rhs=ls[sc][:, :],
                        start=(sc == 0),
                        stop=False,
                    )
                nc.tensor.matmul(
                    out=bg[:, :],
                    lhsT=ones_sb[0:1, 0:128],
                    rhs=dbias_sb[0:1, :],
                    start=False,
                    stop=True,
                )
                bg_sb = singles.tile([128, NREL], f32, name=f"bgs{bc}")
                nc.vector.tensor_copy(out=bg_sb[:, :], in_=bg[:, :])
                nc.sync.dma_start(out=cc_in[bc * 128:(bc + 1) * 128, :], in_=bg_sb[:, :])

            if debug:
                nc.sync.dma_start(out=dbg_bag[:, :], in_=cc_in[:, :])
            nc.gpsimd.collective_compute(
                "AllReduce",
                mybir.AluOpType.add,
                replica_groups=[list(range(NCORES))],
                ins=[cc_in.opt()],
                outs=[cc_out.opt()],
            )

            # softmax over the 53 relations, all 256 bags (2 chunks)
            for bc in range(2):
                t = singles.tile([128, NREL], f32, name=f"sm{bc}")
                nc.sync.dma_start(out=t[:, :], in_=cc_out[bc * 128:(bc + 1) * 128, :])
                nmax = singles.tile([128, 1], f32, name=f"nmax{bc}")
                nc.vector.reduce_max(out=nmax[:, :], in_=t[:, :], axis=AX.X, negate=True)
                ex = singles.tile([128, NREL], f32, name=f"ex{bc}")
                nc.scalar.activation(
                    out=ex[:, :], in_=t[:, :], func=AF.Exp, bias=nmax[:, :], scale=1.0
                )
                ssum = singles.tile([128, 1], f32, name=f"ssum{bc}")
                nc.vector.reduce_sum(out=ssum[:, :], in_=ex[:, :], axis=AX.X)
                rcp = singles.tile([128, 1], f32, name=f"rcp{bc}")
                nc.vector.reciprocal(out=rcp[:, :], in_=ssum[:, :])
                res = singles.tile([128, NREL], f32, name=f"res{bc}")
                nc.vector.tensor_scalar_mul(res[:, :], ex[:, :], rcp[:, :])
                nc.sync.dma_start(out=out_d[bc * 128:(bc + 1) * 128, :], in_=res[:, :])

    nc.compile()
    return nc


def _get_program():
    global _PROGRAM
    if _PROGRAM is None:
        _PROGRAM = _build_program()
    return _PROGRAM


def _pad_edge(a):
    return np.concatenate([a[:, :1], a, a[:, -1:]], axis=1)


def _col_layout(padded, fill=0):
    """[NS, LP] -> per-core column stream [NBLK, BLK_COLS] (pad cols = fill)."""
    a = padded.reshape(NBLK, SG_PER_BLK, SGS * LP)
    out = np.full((NBLK, SG_PER_BLK, SG_COLS), fill, a.dtype)
    out[:, :, :SGS * LP] = a
    return out.reshape(NBLK, BLK_COLS)


def _token_layout(padded):
    """[NS, LP] int32 -> indirect-gather index layout [128, NBLK*32].

    idx[p, blk*32+t] = column stream value at block col t*128+p."""
    flat = _col_layout(padded, 0).reshape(NBLK, TILES_PER_BLK, 128)
    return flat.transpose(2, 0, 1).reshape(128, NBLK * TILES_PER_BLK)


def prepare_in_maps(**inputs):
    sentences = np.asarray(inputs["sentences"]).astype(np.int32)
    pos1 = np.asarray(inputs["pos1"]).astype(np.int32)
    pos2 = np.asarray(inputs["pos2"]).astype(np.int32)
    masks = np.asarray(inputs["masks"]).astype(np.float32)
    bag_ids = np.asarray(inputs["bag_ids"]).astype(np.int64)
    word_emb = np.asarray(inputs["word_emb"]).astype(np.float32)
    pf1_emb = np.asarray(inputs["pf1_emb"]).astype(np.float32)
    pf2_emb = np.asarray(inputs["pf2_emb"]).astype(np.float32)
    conv_w = np.asarray(inputs["conv_w"]).astype(np.float32)
    conv_b = np.asarray(inputs["conv_b"]).astype(np.float32)
    dense_w = np.asarray(inputs["dense_w"]).astype(np.float32)
    dense_b = np.asarray(inputs["dense_b"]).astype(np.float32)

    # ---- shared (replicated) parameter prep ----
    wemb_q = word_emb.astype(BF16)

    # conv weight pair layout [128ch, fc, pair, view, fw]; chunk-2 views get
    # 54 real channel rows, plus +BMASK at the mask row (54) in the tap-1
    # view only.  Packed into one DRAM tensor: pairs then the (2,2) single.
    def _wview(fc, v):
        f0, fw = FCH[fc]
        c, k = v
        if c == 3:
            c, k = 2, 1  # replica serves chunk2 tap 1
        out = np.zeros((128, 128), np.float32)
        nch = 128 if c < 2 else WD + 2 * PD - 256
        out[:nch, :fw] = conv_w[f0:f0 + fw, c * 128:c * 128 + nch, k].T
        if c == 2 and k == 1:
            out[54, :fw] = BMASK
        return out

    wtv = np.zeros((128, 2, 9, 128), np.float32)
    for fc in range(2):
        for v in range(9):
            wtv[:, fc, v] = _wview(fc, (v // 3, v % 3))
    wtp_packed = np.zeros((128, 2560), np.float32)
    wtp_packed[:, :2304] = wtv.reshape(128, 2304)
    wtp_packed = wtp_packed.astype(FP8)

    selb = np.zeros((4, 2, 128), np.float32)
    selb[1, 0, :] = BMASK   # phase j=1 increment rides mask row 1
    selb[2, 1, :] = BMASK   # phase j=2 increment rides mask row 2
    selb = selb.astype(FP8)

    dwt = np.zeros((128, 6 * NREL), np.float32)
    for j in range(3):
        for fc, (f0, fw) in enumerate(FCH):
            dwt[:fw, (j * 2 + fc) * NREL:(j * 2 + fc + 1) * NREL] = \
                dense_w[:, j * NF + f0:j * NF + f0 + fw].T
    dwt = dwt.astype(BF16)

    actb = np.zeros((128, 2), np.float32)
    for fc, (f0, fw) in enumerate(FCH):
        actb[:fw, fc] = conv_b[f0:f0 + fw] - BMASK

    dbias = (dense_b / NCORES).reshape(1, NREL).astype(BF16)
    id16 = np.eye(128, dtype=BF16)

    counts = np.bincount(bag_ids, minlength=NBAGS).astype(np.float32)
    counts = np.maximum(counts, 1.0)

    # ---- per-core prep ----
    in_maps = []
    for r in range(NCORES):
        sl = slice(r * NS, (r + 1) * NS)
        idxw = _token_layout(_pad_edge(sentences[sl])).astype(np.int32)

        m = masks[sl]  # [256, 3, 120]
        md = np.stack([m[:, 0], m[:, 1] - m[:, 0], m[:, 2] - m[:, 1]], axis=1)

        p1p = _pad_edge(pos1[sl])
        p2p = _pad_edge(pos2[sl])
        pfv = np.concatenate([pf1_emb[p1p], pf2_emb[p2p]], axis=2)  # [NS, LP, 10]
        xpf = np.zeros((NBLK, 11, BLK_COLS), np.float32)
        for d in range(2 * PD):
            xpf[:, d, :] = _col_layout(pfv[:, :, d].reshape(NS, LP), 0.0)
        # mask m0 row, center-tap aligned (column t+1 within each sentence)
        mrow = np.zeros((NS, LP), np.float32)
        mrow[:, 1:L + 1] = md[:, 0, :]
        xpf[:, 10, :] = _col_layout(mrow, 0.0)
        xpf = xpf.astype(FP8)

        # rows 1,2 hold the phase increments (m1-m0, m2-m1) in the padded
        # 122-col-per-sentence subgroup layout; selb picks partition j+1
        masksd = np.zeros((NBLK, 4, BLK_COLS), np.float32)
        for j in (1, 2):
            mdpad = np.zeros((NS, LP), np.float32)
            mdpad[:, :L] = md[:, j, :]
            masksd[:, j, :] = _col_layout(mdpad, 0.0)
        masksd = masksd.astype(FP8)

        bags = bag_ids[sl]
        snorm = np.zeros((NS, NBAGS), np.float32)
        snorm[np.arange(NS), bags] = 1.0 / counts[bags]
        snorm = snorm.astype(BF16)

        in_maps.append({
            "wemb": wemb_q,
            "idxw": idxw,
            "xpf": xpf,
            "masksd": masksd,
            "snorm": snorm,
            "wtp": wtp_packed,
            "selb": selb.reshape(4, -1),
            "dwt": dwt,
            "actb": actb,
            "dbias": dbias,
            "id16d": id16,
        })
    return in_maps


def kernel(**inputs):
    in_maps = prepare_in_maps(**inputs)
    nc = _get_program()
    from concourse.bass_utils import run_bass_kernel_spmd

    trace = bool(int(os.environ.get("KERNEL_TRACE", "0")))
    res = run_bass_kernel_spmd(
        nc, in_maps, core_ids=list(range(NCORES)), trace=trace
    )
    global LAST_RESULT
    LAST_RESULT = res
    return res.results[0]["out"].astype(np.float32)


if __name__ == "__main__":
    d = np.load("/root/problem/ref_inputs.npz")
    out = kernel(**{k: d[k] for k in d.files})
    print("out", out.shape, out.dtype)
    ref = np.load("/root/problem/ref_out.npy")
    err = np.abs(out - ref).max() / np.abs(ref).max()
    print("Relative error:", err)


# revision 34
# speedup vs baseline: 1.2394x; 1.0042x over previous
"""Trainium2 Bass kernel for the PCNN (piecewise-CNN) bag-classification model.

v2b pipeline (per NeuronCore, data-parallel over sentences, 256/core):
  1. ONE batched indirect-DMA gather per 32-sentence block (4096 fp8 embedding
     rows; 8 gpsimd instructions/core vs 256 -> SWDGE fixed cost ~19us not 280)
  2. PE transposes (fp8) -> channel-major xcq [128, 4, 4112]; chunk 3 is a
     +15-column-shifted replica of chunk 2 (SBUF->SBUF DMA) so its tap views
     are 16-byte aligned for DoubleRow pairing
  3. conv1d(k=3, edge-pad) as 4 fp8 DoubleRow pair-matmuls (256-deep
     contraction, 0.5 cyc/row) + 1 plain fp8 matmul per (subgroup, filter
     chunk); +128*mask piece-0 bias rides the center-tap view's row 54
  4. PCNN piecewise max-pool: two fp8 rank-1 increment matmuls (pieces 1,2);
     DVE reduce_max reads both filter-chunk PSUM banks in one op
  5. ReLU(max-128+conv_b), dense to 53 logits, bag segment-mean matmul
  6. ReduceScatter(add) over 8 cores -> each core softmaxes its 32 bags,
     host concatenates the 8 x [32,53] slices
"""

import os
import sys

for _p in ("/opt/trn_rl_repo",):
    if _p not in sys.path:
        sys.path.insert(0, _p)

import numpy as np
import ml_dtypes

# ---------------- problem constants (hardcoded per spec) ----------------
N = 2048          # total sentences
L = 120           # max sentence length
LP = 122          # edge-padded length
NCORES = 8
NS = N // NCORES  # 256 sentences per core
BLK = 32          # sentences per block
NBLK = NS // BLK  # 8 blocks
SGS = 4           # sentences per matmul subgroup
SG_PER_BLK = BLK // SGS          # 8
SG_COLS = 512                    # padded columns per subgroup (4*122=488 real)
BLK_COLS = SG_PER_BLK * SG_COLS  # 4096
TILES_PER_BLK = BLK_COLS // 128  # 32
NF = 230
NREL = 53
NBAGS = 256
BAGS_PER_CORE = NBAGS // NCORES  # 32
VOCAB = 100000
WD = 300
PD = 5
ELEM = 300        # gathered bf16 row length == table row stride (indirect
                  # gather's index coefficient comes from the src shape)
BMASK = 128.0     # piecewise-pool mask bias (exact in fp8/bf16)
FCH = [(0, 128), (128, 102)]     # filter chunks
CCH = [(0, 128), (128, 128), (256, 44)]  # word-channel transpose chunks
CSTR = 4112       # xcq chunk stride (4096 cols + room for the +15 replica)
RSHIFT = 15
# DoubleRow k-tile pair views (chunk, tap); pair-dim step must be %16 fp8
# bytes, so taps pair across chunks (step 4112) and chunk2 tap1 reads the
# +15-shifted replica in chunk slot 3 (step 4128). (2,2) runs as a plain
# fp8 single. The PCNN mask row (channel 310 = chunk2 partition 54, stored
# center-tap aligned) gets weight +BMASK only in the tap-1 view.
PAIRS = [((0, 0), (1, 0)), ((0, 1), (1, 1)), ((0, 2), (1, 2)),
         ((2, 0), (3, 1))]
SINGLE = (2, 2)

BF16 = ml_dtypes.bfloat16
FP8 = ml_dtypes.float8_e4m3

_PROGRAM = None
LAST_RESULT = None


def _view_off(v):
    """fp8 free-dim offset of a k-tile view within xcq [128, 4, CSTR]."""
    c, k = v
    if c == 3:  # chunk2 replica, shifted by RSHIFT
        return 3 * CSTR + RSHIFT + k
    return c * CSTR + k


def _build_program():
    import concourse.bass as bass
    import concourse.mybir as mybir
    import concourse.tile as tile
    from concourse import bacc

    f32 = mybir.dt.float32
    bf16 = mybir.dt.bfloat16
    fp8 = mybir.dt.float8e4
    i32 = mybir.dt.int32
    AF = mybir.ActivationFunctionType
    AX = mybir.AxisListType
    PM = mybir.MatmulPerfMode

    nc = bacc.Bacc(
        "TRN2",
        target_bir_lowering=False,
        debug=False,
        num_devices=NCORES,
    )

    # ------------- external I/O -------------
    wemb = nc.dram_tensor("wemb", [VOCAB, ELEM], bf16, kind="ExternalInput").ap()
    idxw_d = nc.dram_tensor("idxw", [128, NBLK * TILES_PER_BLK], i32,
                            kind="ExternalInput").ap()
    xpf_d = nc.dram_tensor("xpf", [NBLK, 11, BLK_COLS], fp8, kind="ExternalInput").ap()
    masks_d = nc.dram_tensor("masksd", [NBLK, 4, BLK_COLS], fp8,
                             kind="ExternalInput").ap()
    snorm_d = nc.dram_tensor("snorm", [NS, NBAGS], bf16, kind="ExternalInput").ap()
    wtp_d = nc.dram_tensor("wtp", [128, 2560], fp8, kind="ExternalInput").ap()
    selb_d = nc.dram_tensor("selb", [4, 2 * 128], fp8, kind="ExternalInput").ap()
    dwt_d = nc.dram_tensor("dwt", [128, 6 * NREL], bf16, kind="ExternalInput").ap()
    actb_d = nc.dram_tensor("actb", [128, 2], f32, kind="ExternalInput").ap()
    dbias_d = nc.dram_tensor("dbias", [1, NREL], bf16, kind="ExternalInput").ap()
    id16_d = nc.dram_tensor("id16d", [128, 128], bf16, kind="ExternalInput").ap()
    out_d = nc.dram_tensor("out", [NBAGS, NREL], f32, kind="ExternalOutput").ap()
    debug = bool(int(os.environ.get("KERNEL_DEBUG", "0")))
    if debug:
        dbg_xcq = nc.dram_tensor("dbg_xcq", [128, 4, CSTR], fp8,
                                 kind="ExternalOutput").ap()
        dbg_pooled = nc.dram_tensor("dbg_pooled", [128, 2, 3, NS], f32,
                                    kind="ExternalOutput").ap()
        dbg_bag = nc.dram_tensor("dbg_bag", [NBAGS, NREL], f32,
                                 kind="ExternalOutput").ap()

    with tile.TileContext(nc) as tc:
        import contextlib

        ctx = contextlib.ExitStack()
        with ctx:
            singles = ctx.enter_context(tc.tile_pool(name="singles", bufs=1))

            # persistent tiles
            wtv_sb = singles.tile([128, 2, 9, 128], fp8, name="wtv")
            selb_sb = singles.tile([4, 2, 128], fp8, name="selb")
            snorm_sb = [singles.tile([128, NBAGS], bf16, name=f"sn{c}") for c in range(2)]
            idxw_sb = singles.tile([128, NBLK * TILES_PER_BLK], i32, name="idx")
            dwt_sb = singles.tile([128, 6 * NREL], bf16)
            actb_sb = singles.tile([128, 2], f32)
            dbias_sb = singles.tile([1, NREL], bf16)
            id16 = singles.tile([128, 128], bf16, name="id16")
            ones_sb = singles.tile([1, 128], bf16)
            pooled = singles.tile([128, 2, 3, NS], f32, name="pool")

            nc.sync.dma_start(out=wtv_sb[:, :, :, :], in_=wtp_d[:, 0:2304])
            nc.sync.dma_start(out=selb_sb[:, :, :], in_=selb_d[:, :])
            for c in range(2):
                nc.sync.dma_start(out=snorm_sb[c][:, :], in_=snorm_d[c * 128:(c + 1) * 128, :])
            nc.sync.dma_start(out=idxw_sb[:, :], in_=idxw_d[:, :])
            nc.sync.dma_start(out=dwt_sb[:, :], in_=dwt_d[:, :])
            nc.sync.dma_start(out=actb_sb[:, :], in_=actb_d[:, :])
            nc.sync.dma_start(out=dbias_sb[:, :], in_=dbias_d[:, :])
            nc.sync.dma_start(out=id16[:, :], in_=id16_d[:, :])
            nc.vector.memset(ones_sb[:, :], 1.0)

            xg_pool = ctx.enter_context(tc.tile_pool(name="xg", bufs=3))
            xcq_pool = ctx.enter_context(tc.tile_pool(name="xcq", bufs=2))
            mask_pool = ctx.enter_context(tc.tile_pool(name="mask", bufs=2))
            tp_psum = ctx.enter_context(tc.tile_pool(name="tp", bufs=2, space="PSUM"))
            cv_psum = ctx.enter_context(tc.tile_pool(name="cv", bufs=3, space="PSUM"))

            NCV = SGS * LP  # 488 contiguous conv output columns per subgroup

            def emit_conv(blk, sg, xcq, ps):
                # fw padded to 128 (zero weight cols) so every PSUM row in both
                # banks is written; out-partition count doesn't affect PE time.
                # chunk2 rows 55:128 are never written, so its views contract
                # only K=55 partitions (same math: weights were 0).
                for fc in range(2):
                    out_ap = ps[0:128, fc, 0:NCV]
                    for v in range(9):
                        c, k = v // 3, v % 3
                        kp = 55 if c == 2 else 128
                        rb = xcq[0:kp, 0, 0:1]
                        off = c * CSTR + k + sg * SG_COLS
                        rhs = bass.AP(tensor=rb.tensor, offset=rb.offset + off,
                                      ap=[rb.ap[0], [1, NCV]])
                        nc.tensor.matmul(
                            out=out_ap,
                            lhsT=wtv_sb[0:kp, fc, v, 0:128],
                            rhs=rhs,
                            start=(v == 0),
                            stop=False,
                            skip_group_check=True,
                        )

            def _rmax(blk, sg, ps, j):
                s0 = blk * BLK + sg * SGS
                pb0 = ps[0:128, 0, 0:1]
                rin = bass.AP(
                    tensor=pb0.tensor, offset=pb0.offset,
                    ap=[pb0.ap[0], [SG_COLS, 2], [LP, SGS], [1, L]],
                )
                pb = pooled[0:128, 0, j, s0:s0 + SGS]
                rout = bass.AP(tensor=pb.tensor, offset=pb.offset,
                               ap=[pb.ap[0], [3 * NS, 2], [1, SGS]])
                nc.vector.reduce_max(out=rout, in_=rin, axis=AX.X)

            def _incs(sg, mask_sb, ps, j):
                for fc in range(2):
                    nc.tensor.matmul(
                        out=ps[0:128, fc, 0:NCV],
                        lhsT=selb_sb[0:4, j, 0:128],
                        rhs=mask_sb[0:4, sg * SG_COLS:sg * SG_COLS + NCV],
                        start=False,
                        stop=(j == 1),
                        skip_group_check=True,
                    )

            # phase half-stages, spread across later convs so the PE never
            # waits on a reduce: H1 = rmax0 + incs1, H2 = rmax1 + incs2 + rmax2
            def emit_jp_h1(blk, sg, mask_sb, ps):
                _rmax(blk, sg, ps, 0)
                _incs(sg, mask_sb, ps, 0)

            def emit_jp_h2(blk, sg, mask_sb, ps):
                _rmax(blk, sg, ps, 1)
                _incs(sg, mask_sb, ps, 1)
                _rmax(blk, sg, ps, 2)

            pending = []
            for blk in range(NBLK):
                # ---- batched gather (token-major, fp8) ----
                xg = xg_pool.tile([128, TILES_PER_BLK, ELEM], bf16, tag="xg")
                for t in range(TILES_PER_BLK):
                    col = blk * TILES_PER_BLK + t
                    nc.gpsimd.indirect_dma_start(
                        out=xg[:, t, 0:WD],
                        out_offset=None,
                        in_=wemb[:, 0:WD],
                        in_offset=bass.IndirectOffsetOnAxis(
                            ap=idxw_sb[:, col:col + 1], axis=0),
                    )
                mask_sb = mask_pool.tile([4, BLK_COLS], fp8, tag="mask")
                nc.sync.dma_start(out=mask_sb[:, :], in_=masks_d[blk, :, :])

                # ---- transpose to channel-major (fp8) ----
                xcq = xcq_pool.tile([128, 4, CSTR], fp8, tag="xcq")
                nc.sync.dma_start(out=xcq[44:55, 2, 0:BLK_COLS], in_=xpf_d[blk, :, :])
                for grp in range(4):  # 8 token-tiles per group
                    for cc, (c0, pw) in enumerate(CCH):
                        tpA = tp_psum.tile([128, 4, 128], bf16, tag="tp", name=f"tpA{cc}")
                        tpB = tp_psum.tile([128, 4, 128], bf16, tag="tp", name=f"tpB{cc}")
                        for t in range(8):
                            ti = grp * 8 + t
                            tgt = tpA if t % 2 == 0 else tpB
                            nc.tensor.transpose(
                                out=tgt[0:pw, t // 2, :],
                                in_=xg[:, ti, c0:c0 + pw],
                                identity=id16[:, :],
                            )
                        for half, tp in ((0, tpA), (1, tpB)):
                            cb = xcq[0:pw, cc, grp * 1024 + half * 128:
                                     grp * 1024 + half * 128 + 1]
                            dst = bass.AP(
                                tensor=cb.tensor, offset=cb.offset,
                                ap=[cb.ap[0], [256, 4], [1, 128]],
                            )
                            nc.scalar.copy(out=dst, in_=tp[0:pw, :, :])

                if debug and blk == 0:
                    nc.sync.dma_start(out=dbg_xcq[:, :, :], in_=xcq[:, :, :])

                for sg in range(SG_PER_BLK):
                    ps = cv_psum.tile([128, 2, SG_COLS], f32, tag="cv",
                                      name=f"cv{blk}_{sg}")
                    emit_conv(blk, sg, xcq, ps)
                    if pending:
                        emit_jp_h1(*pending[-1])
                    if len(pending) >= 2:
                        emit_jp_h2(*pending.pop(0))
                    pending.append((blk, sg, mask_sb, ps))

            emit_jp_h1(*pending[-1])
            while pending:
                emit_jp_h2(*pending.pop(0))

            # ---------------- tail ----------------
            if debug:
                nc.sync.dma_start(out=dbg_pooled[:, :, :, :], in_=pooled[:, :, :, :])
            pr = [singles.tile([128, 3, NS], bf16, name=f"pr{c}") for c in range(2)]
            for fc in range(2):
                nc.scalar.activation(
                    out=pr[fc][:, :, :],
                    in_=pooled[:, fc, :, :],
                    func=AF.Relu,
                    bias=actb_sb[:, fc:fc + 1],
                    scale=1.0,
                )

            # dense: logitsT [53, 256] = sum_{j,fc} dwt[(j,fc)].T @ pr
            lg_ps = cv_psum.tile([NREL, NS], f32, tag="cv", name="lgps")
            nmm = 0
            for j in range(3):
                for fc, (f0, fw) in enumerate(FCH):
                    nc.tensor.matmul(
                        out=lg_ps[:, :],
                        lhsT=dwt_sb[0:fw, (j * 2 + fc) * NREL:(j * 2 + fc + 1) * NREL],
                        rhs=pr[fc][0:fw, j, :],
                        start=(nmm == 0),
                        stop=(nmm == 5),
                    )
                    nmm += 1
            lg_sb = singles.tile([NREL, NS], bf16)
            nc.vector.tensor_copy(out=lg_sb[:, :], in_=lg_ps[:, :])

            # transpose logits -> per-sentence rows [256, 53]
            ls = [singles.tile([128, NREL], bf16, name=f"ls{c}") for c in range(2)]
            for sc in range(2):
                ltp = cv_psum.tile([128, NREL], bf16, tag="cv", name="ltp")
                nc.tensor.transpose(
                    out=ltp[0:128, 0:NREL],
                    in_=lg_sb[:, sc * 128:(sc + 1) * 128],
                    identity=id16[0:NREL, 0:NREL],
                )
                nc.vector.tensor_copy(out=ls[sc][:, :], in_=ltp[0:128, 0:NREL])

            # bag aggregation (+ dense bias/8), full 256 bags of partials
            cc_dram = ctx.enter_context(tc.tile_pool(name="ccd", bufs=1, space="DRAM"))
            cc_in = cc_dram.tile([NBAGS, NREL], f32)
            cc_out = cc_dram.tile([NBAGS, NREL], f32)
            for bc in range(2):
                bg = cv_psum.tile([128, NREL], f32, tag="cv", name="bg")
                for sc in range(2):
                    nc.tensor.matmul(
                        out=bg[:, :],
                        lhsT=snorm_sb[sc][:, bc * 128:(bc + 1) * 128],
                        rhs=ls[sc][:, :],
                        start=(sc == 0),
                        stop=False,
                    )
                nc.tensor.matmul(
                    out=bg[:, :],
                    lhsT=ones_sb[0:1, 0:128],
                    rhs=dbias_sb[0:1, :],
                    start=False,
                    stop=True,
                )
                bg_sb = singles.tile([128, NREL], f32, name=f"bgs{bc}")
                nc.vector.tensor_copy(out=bg_sb[:, :], in_=bg[:, :])
                nc.sync.dma_start(out=cc_in[bc * 128:(bc + 1) * 128, :], in_=bg_sb[:, :])

            if debug:
                nc.sync.dma_start(out=dbg_bag[:, :], in_=cc_in[:, :])
            nc.gpsimd.collective_compute(
                "AllReduce",
                mybir.AluOpType.add,
                replica_groups=[list(range(NCORES))],
                ins=[cc_in.opt()],
                outs=[cc_out.opt()],
            )

            # softmax over the 53 relations, all 256 bags (2 chunks)
            for bc in range(2):
                t = singles.tile([128, NREL], f32, name=f"sm{bc}")
                nc.sync.dma_start(out=t[:, :], in_=cc_out[bc * 128:(bc + 1) * 128, :])
                nmax = singles.tile([128, 1], f32, name=f"nmax{bc}")
                nc.vector.reduce_max(out=nmax[:, :], in_=t[:, :], axis=AX.X, negate=True)
                ex = singles.tile([128, NREL], f32, name=f"ex{bc}")
                nc.scalar.activation(
                    out=ex[:, :], in_=t[:, :], func=AF.Exp, bias=nmax[:, :], scale=1.0
                )
                ssum = singles.tile([128, 1], f32, name=f"ssum{bc}")
                nc.vector.reduce_sum(out=ssum[:, :], in_=ex[:, :], axis=AX.X)
                rcp = singles.tile([128, 1], f32, name=f"rcp{bc}")
                nc.vector.reciprocal(out=rcp[:, :], in_=ssum[:, :])
                res = singles.tile([128, NREL], f32, name=f"res{bc}")
                nc.vector.tensor_scalar_mul(res[:, :], ex[:, :], rcp[:, :])
                nc.sync.dma_start(out=out_d[bc * 128:(bc + 1) * 128, :], in_=res[:, :])

    nc.compile()
    return nc


def _get_program():
    global _PROGRAM
    if _PROGRAM is None:
        _PROGRAM = _build_program()
    return _PROGRAM


def _pad_edge(a):
    return np.concatenate([a[:, :1], a, a[:, -1:]], axis=1)


def _col_layout(padded, fill=0):
    """[NS, LP] -> per-core column stream [NBLK, BLK_COLS] (pad cols = fill)."""
    a = padded.reshape(NBLK, SG_PER_BLK, SGS * LP)
    out = np.full((NBLK, SG_PER_BLK, SG_COLS), fill, a.dtype)
    out[:, :, :SGS * LP] = a
    return out.reshape(NBLK, BLK_COLS)


def _token_layout(padded):
    """[NS, LP] int32 -> indirect-gather index layout [128, NBLK*32].

    idx[p, blk*32+t] = column stream value at block col t*128+p."""
    flat = _col_layout(padded, 0).reshape(NBLK, TILES_PER_BLK, 128)
    return flat.transpose(2, 0, 1).reshape(128, NBLK * TILES_PER_BLK)


def prepare_in_maps(**inputs):
    sentences = np.asarray(inputs["sentences"]).astype(np.int32)
    pos1 = np.asarray(inputs["pos1"]).astype(np.int32)
    pos2 = np.asarray(inputs["pos2"]).astype(np.int32)
    masks = np.asarray(inputs["masks"]).astype(np.float32)
    bag_ids = np.asarray(inputs["bag_ids"]).astype(np.int64)
    word_emb = np.asarray(inputs["word_emb"]).astype(np.float32)
    pf1_emb = np.asarray(inputs["pf1_emb"]).astype(np.float32)
    pf2_emb = np.asarray(inputs["pf2_emb"]).astype(np.float32)
    conv_w = np.asarray(inputs["conv_w"]).astype(np.float32)
    conv_b = np.asarray(inputs["conv_b"]).astype(np.float32)
    dense_w = np.asarray(inputs["dense_w"]).astype(np.float32)
    dense_b = np.asarray(inputs["dense_b"]).astype(np.float32)

    # ---- shared (replicated) parameter prep ----
    wemb_q = word_emb.astype(BF16)

    # conv weight pair layout [128ch, fc, pair, view, fw]; chunk-2 views get
    # 54 real channel rows, plus +BMASK at the mask row (54) in the tap-1
    # view only.  Packed into one DRAM tensor: pairs then the (2,2) single.
    def _wview(fc, v):
        f0, fw = FCH[fc]
        c, k = v
        if c == 3:
            c, k = 2, 1  # replica serves chunk2 tap 1
        out = np.zeros((128, 128), np.float32)
        nch = 128 if c < 2 else WD + 2 * PD - 256
        out[:nch, :fw] = conv_w[f0:f0 + fw, c * 128:c * 128 + nch, k].T
        if c == 2 and k == 1:
            out[54, :fw] = BMASK
        return out

    wtv = np.zeros((128, 2, 9, 128), np.float32)
    for fc in range(2):
        for v in range(9):
            wtv[:, fc, v] = _wview(fc, (v // 3, v % 3))
    wtp_packed = np.zeros((128, 2560), np.float32)
    wtp_packed[:, :2304] = wtv.reshape(128, 2304)
    wtp_packed = wtp_packed.astype(FP8)

    selb = np.zeros((4, 2, 128), np.float32)
    selb[1, 0, :] = BMASK   # phase j=1 increment rides mask row 1
    selb[2, 1, :] = BMASK   # phase j=2 increment rides mask row 2
    selb = selb.astype(FP8)

    dwt = np.zeros((128, 6 * NREL), np.float32)
    for j in range(3):
        for fc, (f0, fw) in enumerate(FCH):
            dwt[:fw, (j * 2 + fc) * NREL:(j * 2 + fc + 1) * NREL] = \
                dense_w[:, j * NF + f0:j * NF + f0 + fw].T
    dwt = dwt.astype(BF16)

    actb = np.zeros((128, 2), np.float32)
    for fc, (f0, fw) in enumerate(FCH):
        actb[:fw, fc] = conv_b[f0:f0 + fw] - BMASK

    dbias = (dense_b / NCORES).reshape(1, NREL).astype(BF16)
    id16 = np.eye(128, dtype=BF16)

    counts = np.bincount(bag_ids, minlength=NBAGS).astype(np.float32)
    counts = np.maximum(counts, 1.0)

    # ---- per-core prep ----
    in_maps = []
    for r in range(NCORES):
        sl = slice(r * NS, (r + 1) * NS)
        idxw = _token_layout(_pad_edge(sentences[sl])).astype(np.int32)

        m = masks[sl]  # [256, 3, 120]
        md = np.stack([m[:, 0], m[:, 1] - m[:, 0], m[:, 2] - m[:, 1]], axis=1)

        p1p = _pad_edge(pos1[sl])
        p2p = _pad_edge(pos2[sl])
        pfv = np.concatenate([pf1_emb[p1p], pf2_emb[p2p]], axis=2)  # [NS, LP, 10]
        xpf = np.zeros((NBLK, 11, BLK_COLS), np.float32)
        for d in range(2 * PD):
            xpf[:, d, :] = _col_layout(pfv[:, :, d].reshape(NS, LP), 0.0)
        # mask m0 row, center-tap aligned (column t+1 within each sentence)
        mrow = np.zeros((NS, LP), np.float32)
        mrow[:, 1:L + 1] = md[:, 0, :]
        xpf[:, 10, :] = _col_layout(mrow, 0.0)
        xpf = xpf.astype(FP8)

        # rows 1,2 hold the phase increments (m1-m0, m2-m1) in the padded
        # 122-col-per-sentence subgroup layout; selb picks partition j+1
        masksd = np.zeros((NBLK, 4, BLK_COLS), np.float32)
        for j in (1, 2):
            mdpad = np.zeros((NS, LP), np.float32)
            mdpad[:, :L] = md[:, j, :]
            masksd[:, j, :] = _col_layout(mdpad, 0.0)
        masksd = masksd.astype(FP8)

        bags = bag_ids[sl]
        snorm = np.zeros((NS, NBAGS), np.float32)
        snorm[np.arange(NS), bags] = 1.0 / counts[bags]
        snorm = snorm.astype(BF16)

        in_maps.append({
            "wemb": wemb_q,
            "idxw": idxw,
            "xpf": xpf,
            "masksd": masksd,
            "snorm": snorm,
            "wtp": wtp_packed,
            "selb": selb.reshape(4, -1),
            "dwt": dwt,
            "actb": actb,
            "dbias": dbias,
            "id16d": id16,
        })
    return in_maps


def kernel(**inputs):
    in_maps = prepare_in_maps(**inputs)
    nc = _get_program()
    from concourse.bass_utils import run_bass_kernel_spmd

    trace = bool(int(os.environ.get("KERNEL_TRACE", "0")))
    res = run_bass_kernel_spmd(
        nc, in_maps, core_ids=list(range(NCORES)), trace=trace
    )
    global LAST_RESULT
    LAST_RESULT = res
    return res.results[0]["out"].astype(np.float32)


if __name__ == "__main__":
    d = np.load("/root/problem/ref_inputs.npz")
    out = kernel(**{k: d[k] for k in d.files})
    print("out", out.shape, out.dtype)
    ref = np.load("/root/problem/ref_out.npy")
    err = np.abs(out - ref).max() / np.abs(ref).max()
    print("Relative error:", err)
